# revision 1
# baseline (speedup 1.0000x reference)
"""CrossAttention Trainium2 kernel (8 NeuronCores, Bass/Tile).

Problem: B=4, Nq=Nk=2048, DIM=1024, HEADS=16, HEAD_DIM=64, fp32.
  q = query @ Wq + bq ; k = key @ Wk + bk ; v = value @ Wv + bv
  attn = softmax(q k^T / 8) ; x = attn v ; out = x @ Wo + bo

Sharding: 8 cores = 4 batches x 2 head-groups (8 heads, 512 channels each).
Each core computes y_partial[b] = (attn-out restricted to its 512 channels) @ Wo_rows;
host sums the two partials per batch and adds bo.

Device design (v3 — ACT-bound pipeline):
  - ACT (scalar) engine does ONLY the softmax exp (the hard floor: 33.5M
    exps/core ~ 255us). Biases / PSUM evacuation run on DVE.
  - QK is row-tiled (K=64 per head -> two heads concurrent in the PE),
    AV col-tiled (M=64 per head -> two heads concurrent), softmax
    denominators from 4x col-tiled M=32 ones-matmuls. Per (quad of 4
    heads, 512-q block, kj tile): PE runs 3 same-mode groups (QK pair x2,
    AV x4, denom x4) against 2x [128,1024] exps on ACT -- mode switches
    between groups drain the PE, so groups are batched by mode.
  - exp(kj) -> QK(kj+1) -> AV(kj) emission keeps ACT gapless: the next
    QK lands on the PE immediately after its st bank is read.
  - q-block boundary: accumulators are evacuated by 3 plain DVE ops
    (x2 copies + recip) hidden under the next block's first exp; the
    per-head normalization muls happen off the critical path (recip rows
    replicated 32->64 partitions via small SBUF->SBUF DMAs).
  - Projections (Q/K/V) and output projection are software-pipelined into
    the attention loop as "filler" PE units, always pumped BEFORE the
    attention instructions that consume them are emitted (in-order engine
    queues -- a late filler behind its consumer would deadlock).
"""

from collections import deque

import numpy as np

import concourse.bass as bass
import concourse.tile as tile
from concourse import bacc, mybir
from concourse.bass_utils import run_bass_kernel_spmd

F32 = mybir.dt.float32
F16 = mybir.dt.float16
EXP = mybir.ActivationFunctionType.Exp

N = 2048          # rows (Nq == Nk)
C = 1024          # model dim
HC = 512          # per-core channels (8 heads x 64)
HD = 64           # head dim
KT_TILES = C // 128   # 8 k-tiles over model dim
NJT = N // 128        # 16 kj tiles
NQB = 4               # q-blocks of 512
SCALE = 0.125         # HEAD_DIM ** -0.5

_CACHE = {}

# Filler pump schedule, (quad, qb) -> (per-kj counts at loop head, mid-loop).
_PUMP_A = {
    (0, 0): [2, 2, 2, 2, 2, 2, 2, 2, 1, 1, 1, 1, 1, 1, 1, 1],
    (0, 1): [2, 1, 1, 1, 1, 1, 1, 1, 0, 0, 0, 0, 0, 0, 0, 0],
    (0, 2): [2, 1, 1, 1, 1, 1, 1, 1, 0, 0, 0, 0, 0, 0, 0, 0],
    (0, 3): [2, 1, 1, 1, 1, 0, 0, 0, 0, 0, 0, 0, 0, 0, 0, 0],
    (1, 0): [2, 1, 0, 0, 0, 0, 0, 0, 0, 0, 0, 0, 0, 0, 0, 0],
    (1, 1): [2, 1, 1, 1, 1, 1, 1, 1, 1, 1, 0, 0, 0, 0, 0, 0],
    (1, 2): [2, 1, 1, 1, 1, 1, 1, 0, 0, 0, 0, 0, 0, 0, 0, 0],
    (1, 3): [2, 1, 1, 1, 1, 1, 1, 0, 0, 0, 0, 0, 0, 0, 0, 0],
}
_PUMP_B = {
    (0, 0): [1, 1, 1, 0, 1, 0, 0, 0, 1, 0, 0, 1, 0, 0, 0, 0],
    (0, 1): [0] * 16,
    (0, 2): [0] * 16,
    (0, 3): [0] * 16,
    (1, 0): [0] * 16,
    (1, 1): [0] * 16,
    (1, 2): [0] * 16,
    (1, 3): [0] * 16,
}


def _build():
    nc = bacc.Bacc("TRN2", target_bir_lowering=False, debug=False)

    # inputs are pre-swizzled host-side so every DMA reads contiguous
    # >=8KB-per-partition lines (straight strided loads only hit ~133GB/s)
    xqT = nc.dram_tensor("xqT", [4, 128, KT_TILES, 512], F16, kind="ExternalInput")
    xkT = nc.dram_tensor("xkT", [4, 128, KT_TILES, 512], F16, kind="ExternalInput")
    xvT = nc.dram_tensor("xvT", [4, 128, KT_TILES, 512], F16, kind="ExternalInput")
    wq = nc.dram_tensor("wq", [128, KT_TILES, HC], F16, kind="ExternalInput")
    wk = nc.dram_tensor("wk", [128, KT_TILES, HC], F16, kind="ExternalInput")
    wv = nc.dram_tensor("wv", [128, KT_TILES, HC], F16, kind="ExternalInput")
    wo = nc.dram_tensor("wo", [128, 4, C], F16, kind="ExternalInput")
    bq = nc.dram_tensor("bq", [HC], F32, kind="ExternalInput")
    bk = nc.dram_tensor("bk", [HC], F32, kind="ExternalInput")
    bv = nc.dram_tensor("bv", [HC], F32, kind="ExternalInput")
    y = nc.dram_tensor("y", [N, C], F32, kind="ExternalOutput")

    with tile.TileContext(nc) as tc:
        with (
            tc.tile_pool(name="persist", bufs=1) as pp,
            tc.tile_pool(name="chunks", bufs=2) as cp,
            tc.tile_pool(name="ptp", bufs=4) as ptp,
            tc.tile_pool(name="yop", bufs=2) as yop,
            tc.tile_pool(name="nrm", bufs=2) as nrm,
            tc.tile_pool(name="stp", bufs=2, space="PSUM") as stp,
            tc.tile_pool(name="accp", bufs=1, space="PSUM") as accp,
            tc.tile_pool(name="scrp", bufs=1, space="PSUM") as scrp,
        ):
            # ---- input DMAs, most-urgent first ----
            chunk = {}
            nload = [0]

            def load_chunk(stream, dram, sl, split=1):
                nload[0] += 1
                t = cp.tile([128, KT_TILES, 512], F16, tag=stream,
                            name=f"{stream}_{sl}_{nload[0]}")
                src = dram[sl]
                # split across partition ranges -> parallel DMA queues
                # (urgent loads on the critical lead-in path)
                for s in range(split):
                    psl = slice(s * 128 // split, (s + 1) * 128 // split)
                    nc.sync.dma_start(t[psl, :, :], src[psl, :, :])
                chunk[(stream, sl)] = t

            load_chunk("xq", xqT, 0, split=2)
            load_chunk("xk", xkT, 0, split=2)
            wq_sb = pp.tile([128, KT_TILES, HC], F16)
            for s in range(2):
                psl = slice(s * 64, s * 64 + 64)
                nc.sync.dma_start(wq_sb[psl, :, :], wq[psl, :, :])
            wk_sb = pp.tile([128, KT_TILES, HC], F16)
            for s in range(2):
                psl = slice(s * 64, s * 64 + 64)
                nc.sync.dma_start(wk_sb[psl, :, :], wk[psl, :, :])
            load_chunk("xv", xvT, 0, split=2)
            wv_sb = pp.tile([128, KT_TILES, HC], F16)
            nc.sync.dma_start(wv_sb[:], wv[:, :, :])
            wo_sb = pp.tile([128, 4, C], F16)
            nc.sync.dma_start(wo_sb[:], wo[:, :, :])
            bq_sb = pp.tile([128, 4], F32)
            nc.sync.dma_start(bq_sb[:], bq.rearrange("(t p) -> p t", p=128))
            bk_sb = pp.tile([128, 4], F32)
            nc.sync.dma_start(bk_sb[:], bk.rearrange("(t p) -> p t", p=128))
            bv_sb = pp.tile([1, HC], F32)
            nc.sync.dma_start(bv_sb[:], bv.rearrange("(o c) -> o c", o=1))
            bv_bc = pp.tile([128, HC], F32)
            nc.gpsimd.partition_broadcast(bv_bc[:], bv_sb[0:1, :])

            # ---- persistent tiles / PE warm-up during initial DMA wait ----
            warm = pp.tile([128, 512], F16)
            nc.vector.memset(warm[:], 0.125)
            wt = scrp.tile([128, 512], F32, tag="ps", name="warm_ps")
            for j in range(12):
                nc.tensor.matmul(wt[:], warm[:, 0:128], warm[:],
                                 start=True, stop=True)

            ones64 = pp.tile([128, 64], F16)
            nc.vector.memset(ones64[:], 1.0)

            # preload the exp ACT table so it doesn't stall attention entry
            exp_dump = pp.tile([1, 32], F32)
            nc.scalar.activation(exp_dump[:], ones64[0:1, 0:32], EXP, scale=0.0)

            QT = pp.tile([128, 4, N], F16)   # [ch-in-tile, qc-tile, q-row]
            KT = pp.tile([128, 4, N], F16)   # same layout as QT
            V = pp.tile([128, NJT, HC], F16)  # [kj-row, kj-tile, channel]
            xT = pp.tile([128, 4, N], F16)   # attention out, [ch, q] layout

            # ---- projection / outproj units (PE fillers) ----
            def qk_unit(dstT, w_sb, b_sb, stream, qc, sl):
                xc = chunk[(stream, sl)]
                ps = scrp.tile([128, 512], F32, tag="ps", name=f"{stream}{qc}_{sl}")
                for k in range(KT_TILES):
                    nc.tensor.matmul(
                        ps[:], w_sb[:, k, qc * 128:(qc + 1) * 128], xc[:, k, :],
                        start=(k == 0), stop=(k == KT_TILES - 1),
                    )
                nc.vector.tensor_scalar_add(
                    dstT[:, qc, sl * 512:(sl + 1) * 512], ps[:], b_sb[:, qc:qc + 1]
                )

            def v_unit(rc, rt):
                kj = rc * 4 + rt
                xc = chunk[("xv", rc)]
                ps = scrp.tile([128, 512], F32, tag="ps", name=f"v_{kj}")
                for k in range(KT_TILES):
                    nc.tensor.matmul(
                        ps[:], xc[:, k, rt * 128:(rt + 1) * 128], wv_sb[:, k, :],
                        start=(k == 0), stop=(k == KT_TILES - 1),
                    )
                nc.vector.tensor_add(V[:, kj, :], ps[:], bv_bc[:])

            ysb_cur = [None]

            def outproj_unit(it, oc, pool_tag=None):
                pool, tag = pool_tag if pool_tag else (scrp, "ps")
                yps = pool.tile([128, 512], F32, tag=tag, name=f"y_{it}_{oc}")
                if oc == 0:
                    ysb_cur[0] = yop.tile([128, C], F32, tag="ysb", name=f"ysb_{it}")
                ysb = ysb_cur[0]
                for ct in range(4):
                    nc.tensor.matmul(
                        yps[:], xT[:, ct, it * 128:(it + 1) * 128],
                        wo_sb[:, ct, oc * 512:(oc + 1) * 512],
                        start=(ct == 0), stop=(ct == 3),
                    )
                nc.vector.tensor_copy(ysb[:, oc * 512:(oc + 1) * 512], yps[:])
                if oc == 1:
                    nc.sync.dma_start(y[it * 128:(it + 1) * 128, :], ysb[:])

            fillers = deque()

            def pump(n):
                for _ in range(n):
                    if not fillers:
                        return
                    fillers.popleft()()

            # ---- prologue: minimal proj so attention can start ----
            with nc.named_scope("prologue"):
                qk_unit(QT, wq_sb, bq_sb, "xq", 0, 0)
                qk_unit(QT, wq_sb, bq_sb, "xq", 1, 0)
                qk_unit(KT, wk_sb, bk_sb, "xk", 0, 0)
                qk_unit(KT, wk_sb, bk_sb, "xk", 1, 0)
                v_unit(0, 0)
                v_unit(0, 1)

            # ---- enqueue A-phase fillers (consumed during quad-0 attn) ----
            fillers.append(lambda: v_unit(0, 2))
            fillers.append(lambda: v_unit(0, 3))
            for rc in (1, 2, 3):
                fillers.append(lambda rc=rc: load_chunk("xk", xkT, rc))
                fillers.append(lambda rc=rc: qk_unit(KT, wk_sb, bk_sb, "xk", 0, rc))
                fillers.append(lambda rc=rc: qk_unit(KT, wk_sb, bk_sb, "xk", 1, rc))
                fillers.append(lambda rc=rc: load_chunk("xv", xvT, rc))
                for rt in range(4):
                    fillers.append(lambda rc=rc, rt=rt: v_unit(rc, rt))
            fillers.append(lambda: load_chunk("xq", xqT, 1))
            fillers.append(lambda: qk_unit(QT, wq_sb, bq_sb, "xq", 0, 1))
            fillers.append(lambda: qk_unit(QT, wq_sb, bq_sb, "xq", 1, 1))
            # (0,1): Q_A qb2 + K_B rc0/rc1 ; (0,2): Q_A qb3 + K_B rc2/rc3
            for qb, rcs in ((2, (0, 1)), (3, (2, 3))):
                fillers.append(lambda qb=qb: load_chunk("xq", xqT, qb))
                fillers.append(lambda qb=qb: qk_unit(QT, wq_sb, bq_sb, "xq", 0, qb))
                fillers.append(lambda qb=qb: qk_unit(QT, wq_sb, bq_sb, "xq", 1, qb))
                for rc in rcs:
                    fillers.append(lambda rc=rc: load_chunk("xk", xkT, rc))
                    fillers.append(lambda rc=rc: qk_unit(KT, wk_sb, bk_sb, "xk", 2, rc))
                    fillers.append(lambda rc=rc: qk_unit(KT, wk_sb, bk_sb, "xk", 3, rc))
            # quad-B Q projections (qb0..3), pumped at (0,3)/(1,0)/(1,1)
            for qb in range(4):
                fillers.append(lambda qb=qb: load_chunk("xq", xqT, qb))
                fillers.append(lambda qb=qb: qk_unit(QT, wq_sb, bq_sb, "xq", 2, qb))
                fillers.append(lambda qb=qb: qk_unit(QT, wq_sb, bq_sb, "xq", 3, qb))

            # ---- attention ----
            def emit_qk_pair(st, c, qb, kj):
                kjsl = slice(kj * 128, (kj + 1) * 128)
                qbsl = slice(qb * 512, (qb + 1) * 512)
                nc.tensor.matmul(
                    st[:, 0:512], KT[0:64, c, kjsl], QT[0:64, c, qbsl],
                    start=True, stop=True, tile_position=(0, 0),
                )
                nc.tensor.matmul(
                    st[:, 512:1024], KT[64:128, c, kjsl], QT[64:128, c, qbsl],
                    start=True, stop=True, tile_position=(64, 0),
                )

            for quad in range(2):
                c0, c1 = 2 * quad, 2 * quad + 1
                ca = quad * 256
                with nc.named_scope(f"attn{quad}"):
                    for qb in range(NQB):
                        pa = _PUMP_A[(quad, qb)]
                        pb = _PUMP_B[(quad, qb)]
                        x_ab = accp.tile([128, 512], F32, tag="xab", name=f"xab_{quad}{qb}")
                        x_cd = accp.tile([128, 512], F32, tag="xcd", name=f"xcd_{quad}{qb}")
                        dd = accp.tile([128, 512], F32, tag="dd", name=f"dd_{quad}{qb}")
                        stAB = stp.tile([128, 1024], F32, tag="st", name=f"st_{quad}{qb}_0ab")
                        stCD = stp.tile([128, 1024], F32, tag="st", name=f"st_{quad}{qb}_0cd")
                        emit_qk_pair(stAB, c0, qb, 0)
                        emit_qk_pair(stCD, c1, qb, 0)
                        for kj in range(NJT):
                            first, last = (kj == 0), (kj == NJT - 1)
                            pump(pa[kj])
                            ptAB = ptp.tile([128, 1024], F16, tag="pt",
                                            name=f"pt_{quad}{qb}{kj}ab")
                            nc.scalar.activation(ptAB[:], stAB[:], EXP, scale=SCALE)
                            ptCD = ptp.tile([128, 1024], F16, tag="pt",
                                            name=f"pt_{quad}{qb}{kj}cd")
                            nc.scalar.activation(ptCD[:], stCD[:], EXP, scale=SCALE)
                            # next kj's QK pairs (one 64x128 mode group);
                            # lands on the PE the moment each st is read
                            if not last:
                                stAB = stp.tile([128, 1024], F32, tag="st",
                                                name=f"st_{quad}{qb}_{kj + 1}ab")
                                emit_qk_pair(stAB, c0, qb, kj + 1)
                                stCD = stp.tile([128, 1024], F32, tag="st",
                                                name=f"st_{quad}{qb}_{kj + 1}cd")
                                emit_qk_pair(stCD, c1, qb, kj + 1)
                            pump(pb[kj])
                            # AV, all 4 heads in one 128x64 mode group; on
                            # the last kj, evacuation DVE ops are interleaved
                            # right behind each accumulator's final matmul so
                            # the banks free up before the next q-block.
                            if last:
                                xu_ab = nrm.tile([128, 512], F32, tag="xuab",
                                                 name=f"xu_{quad}{qb}a")
                                xu_cd = nrm.tile([128, 512], F32, tag="xucd",
                                                 name=f"xu_{quad}{qb}c")
                                r32 = nrm.tile([128, 512], F32, tag="r32",
                                               name=f"r_{quad}{qb}")
                            nc.tensor.matmul(
                                x_ab[0:64, :], V[:, kj, ca:ca + 64], ptAB[:, 0:512],
                                start=first, stop=last, tile_position=(0, 0),
                            )
                            nc.tensor.matmul(
                                x_ab[64:128, :], V[:, kj, ca + 64:ca + 128],
                                ptAB[:, 512:1024],
                                start=first, stop=last, tile_position=(0, 64),
                            )
                            if last:
                                nc.vector.tensor_copy(xu_ab[:], x_ab[:])
                            nc.tensor.matmul(
                                x_cd[0:64, :], V[:, kj, ca + 128:ca + 192],
                                ptCD[:, 0:512],
                                start=first, stop=last, tile_position=(0, 0),
                            )
                            nc.tensor.matmul(
                                x_cd[64:128, :], V[:, kj, ca + 192:ca + 256],
                                ptCD[:, 512:1024],
                                start=first, stop=last, tile_position=(0, 64),
                            )
                            if last:
                                nc.vector.tensor_copy(xu_cd[:], x_cd[:])
                            # softmax denominators: 4 heads in one 128x32
                            # mode group (disjoint partition ranges, 1 bank)
                            for h, pt_sl in enumerate(
                                (ptAB[:, 0:512], ptAB[:, 512:1024],
                                 ptCD[:, 0:512], ptCD[:, 512:1024])
                            ):
                                nc.tensor.matmul(
                                    dd[32 * h:32 * h + 32, :], ones64[:, 0:32],
                                    pt_sl, start=first, stop=last,
                                    tile_position=(0, 32 * h),
                                )
                            if last:
                                nc.vector.reciprocal_approx_fast(r32[:], dd[:])
                        # ...then normalize off the critical path: replicate
                        # the recips 32->64 partitions, multiply into xT
                        RAB = nrm.tile([128, 512], F32, tag="RAB", name=f"Rab_{quad}{qb}")
                        RCD = nrm.tile([128, 512], F32, tag="RCD", name=f"Rcd_{quad}{qb}")
                        for h in range(2):
                            src = r32[32 * h:32 * h + 32, :]
                            nc.sync.dma_start(RAB[64 * h:64 * h + 32, :], src)
                            nc.sync.dma_start(RAB[64 * h + 32:64 * h + 64, :], src)
                            src = r32[64 + 32 * h:64 + 32 * h + 32, :]
                            nc.sync.dma_start(RCD[64 * h:64 * h + 32, :], src)
                            nc.sync.dma_start(RCD[64 * h + 32:64 * h + 64, :], src)
                        qbsl = slice(qb * 512, (qb + 1) * 512)
                        nc.vector.tensor_mul(xT[:, c0, qbsl], xu_ab[:], RAB[:])
                        nc.vector.tensor_mul(xT[:, c1, qbsl], xu_cd[:], RCD[:])
                        # enqueue outproj for this qb once quad B's half
                        # exists; the final block's units spread across the
                        # freed accumulator banks so the tail pipelines
                        if quad == 1:
                            tail_tags = [(scrp, "ps"), (accp, "xab"),
                                         (accp, "xcd"), (accp, "dd")]
                            for i, (it, oc) in enumerate(
                                (it, oc)
                                for it in range(qb * 4, qb * 4 + 4)
                                for oc in range(2)
                            ):
                                pt_sel = tail_tags[i % 4] if qb == 3 else None
                                fillers.append(
                                    lambda it=it, oc=oc, pt_sel=pt_sel:
                                        outproj_unit(it, oc, pt_sel))

            # ---- drain remaining fillers (outproj tail) ----
            with nc.named_scope("tail"):
                pump(len(fillers))

    nc.finalize()
    return nc


def _get_nc():
    if "nc" not in _CACHE:
        _CACHE["nc"] = _build()
    return _CACHE["nc"]


def _swz_x(x):
    # x [N, C] -> x^T [C, N] chunked as [4 row-chunks, 128 part, 8 ktile, 512]
    xT = np.asarray(x, np.float32).T.astype(np.float16)          # [C, N]
    return np.ascontiguousarray(
        xT.reshape(KT_TILES, 128, 4, 512).transpose(2, 1, 0, 3))


def _swz_w(w):
    # w [C, HC_slice] -> [128 part, 8 ktile, cols]
    w = np.asarray(w, np.float32).astype(np.float16)
    return np.ascontiguousarray(
        w.reshape(KT_TILES, 128, w.shape[1]).transpose(1, 0, 2))


def _make_in_maps(query, key, value, Wq, bq, Wk, bk, Wv, bv, Wo):
    f = np.float32
    in_maps = []
    for core in range(8):
        b, hg = divmod(core, 2)
        sl = slice(hg * HC, (hg + 1) * HC)
        in_maps.append({
            "xqT": _swz_x(query[b]),
            "xkT": _swz_x(key[b]),
            "xvT": _swz_x(value[b]),
            "wq": _swz_w(np.asarray(Wq, f)[:, sl]),
            "wk": _swz_w(np.asarray(Wk, f)[:, sl]),
            "wv": _swz_w(np.asarray(Wv, f)[:, sl]),
            "wo": np.ascontiguousarray(
                np.asarray(Wo, f)[sl, :].astype(np.float16)
                .reshape(4, 128, C).transpose(1, 0, 2)),
            "bq": np.ascontiguousarray(np.asarray(bq, f)[sl]),
            "bk": np.ascontiguousarray(np.asarray(bk, f)[sl]),
            "bv": np.ascontiguousarray(np.asarray(bv, f)[sl]),
        })
    return in_maps


def _run(inputs, trace=False, **kwargs):
    nc = _get_nc()
    in_maps = _make_in_maps(
        inputs["query"], inputs["key"], inputs["value"],
        inputs["Wq"], inputs["bq"], inputs["Wk"], inputs["bk"],
        inputs["Wv"], inputs["bv"], inputs["Wo"],
    )
    res = run_bass_kernel_spmd(nc, in_maps, core_ids=list(range(8)), trace=trace, **kwargs)
    bo = np.asarray(inputs["bo"], np.float32)
    out = np.empty((4, N, C), np.float32)
    for b in range(4):
        out[b] = res.results[2 * b]["y"] + res.results[2 * b + 1]["y"] + bo
    return out, res


def kernel(**inputs) -> np.ndarray:
    out, _ = _run(inputs, trace=False)
    return out



# revision 3
# speedup vs baseline: 1.0727x; 1.0727x over previous
"""CrossAttention Trainium2 kernel (8 NeuronCores, Bass/Tile).

Problem: B=4, Nq=Nk=2048, DIM=1024, HEADS=16, HEAD_DIM=64, fp32.
  q = query @ Wq + bq ; k = key @ Wk + bk ; v = value @ Wv + bv
  attn = softmax(q k^T / 8) ; x = attn v ; out = x @ Wo + bo

Sharding: 8 cores = 4 batches x 2 head-groups (8 heads, 512 channels each).
Each core computes y_partial[b] = (attn-out restricted to its 512 channels) @ Wo_rows;
host sums the two partials per batch and adds bo.

Device design (v4):
  - ACT (scalar) does only the softmax exps (256 x [128,1024], ~1us each).
  - Softmax denominators: DVE accumulates ptsum += pt (fp16) per kj tile;
    one 4-head col-tiled ones-matmul per q-block turns ptsum into the
    denominators (frees ~43us of PE vs per-kj ones-matmuls, and frees the
    dd PSUM bank so filler scratch can double-buffer).
  - Per-iter PE order is split into AB-side / CD-side halves so the PE
    never head-of-line blocks on the not-yet-finished CD exp:
      QK-AB(kj+1) | AV-ab(kj) | [pump] | QK-CD(kj+1) | AV-cd(kj) | [pump]
  - DMA priority: biases first, then the minimal first-exp set
    (wq/wk qc0+qc1 slices, xq0, xk0), then wv/xv0; bulk chunks stream in
    as fillers. Weights are swizzled [128, qc, kt, 128] host-side so a
    per-qc slice is one contiguous 2KB/partition DMA.
  - Long PE warmup keeps HAM at 8/8 through the DMA lead-in.
  - Normalization: reciprocal rows are replicated 32->64 partitions with
    gpsimd.partition_broadcast (no sync-queue DMAs in the chain).
  - y is written fp16 (halves output DMA); host sums partials in fp32.
"""

from collections import deque

import numpy as np

import concourse.bass as bass
import concourse.tile as tile
from concourse import bacc, mybir
from concourse.bass_utils import run_bass_kernel_spmd

F32 = mybir.dt.float32
F16 = mybir.dt.float16
EXP = mybir.ActivationFunctionType.Exp

N = 2048          # rows (Nq == Nk)
C = 1024          # model dim
HC = 512          # per-core channels (8 heads x 64)
HD = 64           # head dim
KT_TILES = C // 128   # 8 k-tiles over model dim
NJT = N // 128        # 16 kj tiles
NQB = 4               # q-blocks of 512
SCALE = 0.125         # HEAD_DIM ** -0.5
WARMUP_MMS = 26       # PE keep-warm matmuls during the DMA lead-in

_CACHE = {}

# Filler pump schedule, (quad, qb) -> per-kj counts.
# _PUMP_A fires at the top of the iter (before QK kj+1: delays next exp,
# use only when the backlog demands it); _PUMP_B fires after AV-cd (cheap).
_PUMP_A = {
    (0, 0): [1, 1, 1, 1, 1, 1, 1, 1, 1, 1, 1, 1, 1, 1, 1, 1],
    (0, 1): [1, 1, 1, 1, 0, 0, 0, 0, 0, 0, 0, 0, 0, 0, 0, 0],
    (0, 2): [0] * 16,
    (0, 3): [0] * 16,
    (1, 0): [0] * 16,
    (1, 1): [1, 1, 0, 0, 0, 0, 0, 0, 0, 0, 0, 0, 0, 0, 0, 0],
    (1, 2): [1, 1, 0, 0, 0, 0, 0, 0, 0, 0, 0, 0, 0, 0, 0, 0],
    (1, 3): [1, 0, 0, 0, 0, 0, 0, 0, 0, 0, 0, 0, 0, 0, 0, 0],
}
_PUMP_B = {
    (0, 0): [1, 1, 1, 1, 1, 1, 1, 1, 1, 1, 1, 1, 1, 1, 1, 1],
    (0, 1): [1, 1, 1, 1, 1, 1, 1, 1, 1, 1, 1, 1, 1, 1, 0, 0],
    (0, 2): [1, 1, 1, 1, 1, 1, 1, 1, 1, 0, 0, 0, 0, 0, 0, 0],
    (0, 3): [1, 1, 1, 1, 1, 1, 0, 0, 0, 0, 0, 0, 0, 0, 0, 0],
    (1, 0): [1, 1, 1, 0, 0, 0, 0, 0, 0, 0, 0, 0, 0, 0, 0, 0],
    (1, 1): [1, 1, 1, 1, 1, 1, 1, 1, 1, 0, 0, 0, 0, 0, 0, 0],
    (1, 2): [1, 1, 1, 1, 1, 1, 1, 1, 1, 0, 0, 0, 0, 0, 0, 0],
    (1, 3): [1, 1, 1, 1, 1, 1, 1, 0, 0, 0, 0, 0, 0, 0, 0, 0],
}


def _build():
    nc = bacc.Bacc("TRN2", target_bir_lowering=False, debug=False)

    # inputs are pre-swizzled host-side so every DMA reads contiguous
    # >=2KB-per-partition lines
    xqT = nc.dram_tensor("xqT", [4, 128, KT_TILES, 512], F16, kind="ExternalInput")
    xkT = nc.dram_tensor("xkT", [4, 128, KT_TILES, 512], F16, kind="ExternalInput")
    xvT = nc.dram_tensor("xvT", [4, 128, KT_TILES, 512], F16, kind="ExternalInput")
    # wq/wk: [part, qc-tile, k-tile, 128] so a qc slice is contiguous
    wq = nc.dram_tensor("wq", [128, 4, KT_TILES, 128], F16, kind="ExternalInput")
    wk = nc.dram_tensor("wk", [128, 4, KT_TILES, 128], F16, kind="ExternalInput")
    wv = nc.dram_tensor("wv", [128, KT_TILES, HC], F16, kind="ExternalInput")
    wo = nc.dram_tensor("wo", [128, 4, C], F16, kind="ExternalInput")
    bq = nc.dram_tensor("bq", [HC], F32, kind="ExternalInput")
    bk = nc.dram_tensor("bk", [HC], F32, kind="ExternalInput")
    bv = nc.dram_tensor("bv", [HC], F32, kind="ExternalInput")
    y = nc.dram_tensor("y", [N, C], F16, kind="ExternalOutput")

    with tile.TileContext(nc) as tc:
        with (
            tc.tile_pool(name="persist", bufs=1) as pp,
            tc.tile_pool(name="chunks", bufs=2) as cp,
            tc.tile_pool(name="ptp", bufs=4) as ptp,
            tc.tile_pool(name="pts", bufs=2) as pts,
            tc.tile_pool(name="yop", bufs=2) as yop,
            tc.tile_pool(name="nrm", bufs=2) as nrm,
            tc.tile_pool(name="stp", bufs=2, space="PSUM") as stp,
            tc.tile_pool(name="accp", bufs=1, space="PSUM") as accp,
            tc.tile_pool(name="scrp", bufs=2, space="PSUM") as scrp,
        ):
            # ---- DMAs, strict priority order ----
            # biases first (tiny; a late bias stalls the first bias-add)
            bq_sb = pp.tile([128, 4], F32)
            nc.sync.dma_start(bq_sb[:], bq.rearrange("(t p) -> p t", p=128))
            bk_sb = pp.tile([128, 4], F32)
            nc.sync.dma_start(bk_sb[:], bk.rearrange("(t p) -> p t", p=128))
            bv_sb = pp.tile([1, HC], F32)
            nc.sync.dma_start(bv_sb[:], bv.rearrange("(o c) -> o c", o=1))

            chunk = {}
            nload = [0]

            def load_chunk(stream, dram, sl, split=1):
                nload[0] += 1
                t = cp.tile([128, KT_TILES, 512], F16, tag=stream,
                            name=f"{stream}_{sl}_{nload[0]}")
                src = dram[sl]
                for s in range(split):
                    psl = slice(s * 128 // split, (s + 1) * 128 // split)
                    nc.sync.dma_start(t[psl, :, :], src[psl, :, :])
                chunk[(stream, sl)] = t

            # critical set for the first exps: wq/wk qc0+qc1, xq0, xk0
            wq_sb = pp.tile([128, 4, KT_TILES, 128], F16)
            nc.sync.dma_start(wq_sb[:, 0], wq[:, 0])
            wk_sb = pp.tile([128, 4, KT_TILES, 128], F16)
            nc.sync.dma_start(wk_sb[:, 0], wk[:, 0])
            load_chunk("xq", xqT, 0, split=2)
            nc.sync.dma_start(wq_sb[:, 1], wq[:, 1])
            nc.sync.dma_start(wk_sb[:, 1], wk[:, 1])
            load_chunk("xk", xkT, 0, split=2)
            # then what AV(kj0..3) needs
            wv_sb = pp.tile([128, KT_TILES, HC], F16)
            nc.sync.dma_start(wv_sb[:], wv[:, :, :])
            load_chunk("xv", xvT, 0, split=2)
            # quad-B weight halves (needed from (0,3) fillers on)
            nc.sync.dma_start(wq_sb[:, 2], wq[:, 2])
            nc.sync.dma_start(wk_sb[:, 2], wk[:, 2])
            nc.sync.dma_start(wq_sb[:, 3], wq[:, 3])
            nc.sync.dma_start(wk_sb[:, 3], wk[:, 3])
            wo_sb = pp.tile([128, 4, C], F16)
            nc.sync.dma_start(wo_sb[:], wo[:, :, :])

            bv_bc = pp.tile([128, HC], F32)
            nc.gpsimd.partition_broadcast(bv_bc[:], bv_sb[0:1, :])

            # ---- persistent tiles / PE warm-up during initial DMA wait ----
            warm = pp.tile([128, 512], F16)
            nc.vector.memset(warm[:], 0.125)
            wt = scrp.tile([128, 512], F32, tag="ps", name="warm_ps")
            for j in range(WARMUP_MMS):
                nc.tensor.matmul(wt[:], warm[:, 0:128], warm[:],
                                 start=True, stop=True)

            ones64 = pp.tile([128, 64], F16)
            nc.vector.memset(ones64[:], 1.0)

            # preload the exp ACT table so it doesn't stall attention entry
            exp_dump = pp.tile([1, 32], F32)
            nc.scalar.activation(exp_dump[:], ones64[0:1, 0:32], EXP, scale=0.0)

            QT = pp.tile([128, 4, N], F16)   # [ch-in-tile, qc-tile, q-row]
            KT = pp.tile([128, 4, N], F16)   # same layout as QT
            V = pp.tile([128, NJT, HC], F16)  # [kj-row, kj-tile, channel]
            xT = pp.tile([128, 4, N], F16)   # attention out, [ch, q] layout

            # ---- projection / outproj units (PE fillers) ----
            # pool_tag rotation for the prologue so consecutive units don't
            # serialize on one scratch bank (attention hasn't started yet,
            # so the accumulator banks are free).
            def qk_unit(dstT, w_sb, b_sb, stream, qc, sl, pool_tag=None):
                pool, tag = pool_tag if pool_tag else (scrp, "ps")
                xc = chunk[(stream, sl)]
                ps = pool.tile([128, 512], F32, tag=tag, name=f"{stream}{qc}_{sl}")
                for k in range(KT_TILES):
                    nc.tensor.matmul(
                        ps[:], w_sb[:, qc, k, :], xc[:, k, :],
                        start=(k == 0), stop=(k == KT_TILES - 1),
                    )
                nc.vector.tensor_scalar_add(
                    dstT[:, qc, sl * 512:(sl + 1) * 512], ps[:], b_sb[:, qc:qc + 1]
                )

            def v_unit(rc, rt, pool_tag=None):
                pool, tag = pool_tag if pool_tag else (scrp, "ps")
                kj = rc * 4 + rt
                xc = chunk[("xv", rc)]
                ps = pool.tile([128, 512], F32, tag=tag, name=f"v_{kj}")
                for k in range(KT_TILES):
                    nc.tensor.matmul(
                        ps[:], xc[:, k, rt * 128:(rt + 1) * 128], wv_sb[:, k, :],
                        start=(k == 0), stop=(k == KT_TILES - 1),
                    )
                nc.vector.tensor_add(V[:, kj, :], ps[:], bv_bc[:])

            ysb_cur = [None]

            def outproj_unit(it, oc, pool_tag=None):
                pool, tag = pool_tag if pool_tag else (scrp, "ps")
                yps = pool.tile([128, 512], F32, tag=tag, name=f"y_{it}_{oc}")
                if oc == 0:
                    ysb_cur[0] = yop.tile([128, C], F16, tag="ysb", name=f"ysb_{it}")
                ysb = ysb_cur[0]
                for ct in range(4):
                    nc.tensor.matmul(
                        yps[:], xT[:, ct, it * 128:(it + 1) * 128],
                        wo_sb[:, ct, oc * 512:(oc + 1) * 512],
                        start=(ct == 0), stop=(ct == 3),
                    )
                nc.vector.tensor_copy(ysb[:, oc * 512:(oc + 1) * 512], yps[:])
                if oc == 1:
                    nc.sync.dma_start(y[it * 128:(it + 1) * 128, :], ysb[:])

            fillers = deque()

            def pump(n):
                for _ in range(n):
                    if not fillers:
                        return
                    fillers.popleft()()

            # ---- prologue: minimal proj so attention can start ----
            with nc.named_scope("prologue"):
                qk_unit(QT, wq_sb, bq_sb, "xq", 0, 0, (accp, "xab"))
                qk_unit(KT, wk_sb, bk_sb, "xk", 0, 0, (accp, "xcd"))
                qk_unit(QT, wq_sb, bq_sb, "xq", 1, 0, (scrp, "ps"))
                qk_unit(KT, wk_sb, bk_sb, "xk", 1, 0, (scrp, "ps"))

            # ---- enqueue fillers (consumed during quad-0 attn) ----
            for rt in range(4):
                fillers.append(lambda rt=rt: v_unit(0, rt))
            for rc in (1, 2, 3):
                fillers.append(lambda rc=rc: load_chunk("xk", xkT, rc))
                fillers.append(lambda rc=rc: qk_unit(KT, wk_sb, bk_sb, "xk", 0, rc))
                fillers.append(lambda rc=rc: qk_unit(KT, wk_sb, bk_sb, "xk", 1, rc))
                fillers.append(lambda rc=rc: load_chunk("xv", xvT, rc))
                for rt in range(4):
                    fillers.append(lambda rc=rc, rt=rt: v_unit(rc, rt))
            fillers.append(lambda: load_chunk("xq", xqT, 1))
            fillers.append(lambda: qk_unit(QT, wq_sb, bq_sb, "xq", 0, 1))
            fillers.append(lambda: qk_unit(QT, wq_sb, bq_sb, "xq", 1, 1))
            # (0,1)/(0,2): remaining quad-A Q + quad-B K projections
            for qb, rcs in ((2, (0, 1)), (3, (2, 3))):
                fillers.append(lambda qb=qb: load_chunk("xq", xqT, qb))
                fillers.append(lambda qb=qb: qk_unit(QT, wq_sb, bq_sb, "xq", 0, qb))
                fillers.append(lambda qb=qb: qk_unit(QT, wq_sb, bq_sb, "xq", 1, qb))
                for rc in rcs:
                    fillers.append(lambda rc=rc: load_chunk("xk", xkT, rc))
                    fillers.append(lambda rc=rc: qk_unit(KT, wk_sb, bk_sb, "xk", 2, rc))
                    fillers.append(lambda rc=rc: qk_unit(KT, wk_sb, bk_sb, "xk", 3, rc))
            # quad-B Q projections (qb0..3), pumped at (0,3)/(1,0)/(1,1)
            for qb in range(4):
                fillers.append(lambda qb=qb: load_chunk("xq", xqT, qb))
                fillers.append(lambda qb=qb: qk_unit(QT, wq_sb, bq_sb, "xq", 2, qb))
                fillers.append(lambda qb=qb: qk_unit(QT, wq_sb, bq_sb, "xq", 3, qb))

            # ---- attention ----
            def emit_qk_pair(st, c, qb, kj):
                kjsl = slice(kj * 128, (kj + 1) * 128)
                qbsl = slice(qb * 512, (qb + 1) * 512)
                nc.tensor.matmul(
                    st[:, 0:512], KT[0:64, c, kjsl], QT[0:64, c, qbsl],
                    start=True, stop=True, tile_position=(0, 0),
                )
                nc.tensor.matmul(
                    st[:, 512:1024], KT[64:128, c, kjsl], QT[64:128, c, qbsl],
                    start=True, stop=True, tile_position=(64, 0),
                )

            for quad in range(2):
                c0, c1 = 2 * quad, 2 * quad + 1
                ca = quad * 256
                with nc.named_scope(f"attn{quad}"):
                    for qb in range(NQB):
                        pa = _PUMP_A[(quad, qb)]
                        pb = _PUMP_B[(quad, qb)]
                        x_ab = accp.tile([128, 512], F32, tag="xab", name=f"xab_{quad}{qb}")
                        x_cd = accp.tile([128, 512], F32, tag="xcd", name=f"xcd_{quad}{qb}")
                        psum_ab = pts.tile([128, 1024], F16, tag="psab",
                                           name=f"psab_{quad}{qb}")
                        psum_cd = pts.tile([128, 1024], F16, tag="pscd",
                                           name=f"pscd_{quad}{qb}")
                        stAB = stp.tile([128, 1024], F32, tag="st", name=f"st_{quad}{qb}_0ab")
                        stCD = stp.tile([128, 1024], F32, tag="st", name=f"st_{quad}{qb}_0cd")
                        emit_qk_pair(stAB, c0, qb, 0)
                        emit_qk_pair(stCD, c1, qb, 0)
                        for kj in range(NJT):
                            first, last = (kj == 0), (kj == NJT - 1)
                            pump(pa[kj])
                            ptAB = ptp.tile([128, 1024], F16, tag="pt",
                                            name=f"pt_{quad}{qb}{kj}ab")
                            nc.scalar.activation(ptAB[:], stAB[:], EXP, scale=SCALE)
                            ptCD = ptp.tile([128, 1024], F16, tag="pt",
                                            name=f"pt_{quad}{qb}{kj}cd")
                            nc.scalar.activation(ptCD[:], stCD[:], EXP, scale=SCALE)
                            # --- AB side: next QK, ptsum, AV ---
                            if not last:
                                stAB = stp.tile([128, 1024], F32, tag="st",
                                                name=f"st_{quad}{qb}_{kj + 1}ab")
                                emit_qk_pair(stAB, c0, qb, kj + 1)
                            if first:
                                nc.vector.tensor_copy(psum_ab[:], ptAB[:])
                            else:
                                nc.vector.tensor_add(psum_ab[:], psum_ab[:], ptAB[:])
                            if last:
                                xu_ab = nrm.tile([128, 512], F32, tag="xuab",
                                                 name=f"xu_{quad}{qb}a")
                                xu_cd = nrm.tile([128, 512], F32, tag="xucd",
                                                 name=f"xu_{quad}{qb}c")
                            nc.tensor.matmul(
                                x_ab[0:64, :], V[:, kj, ca:ca + 64], ptAB[:, 0:512],
                                start=first, stop=last, tile_position=(0, 0),
                            )
                            nc.tensor.matmul(
                                x_ab[64:128, :], V[:, kj, ca + 64:ca + 128],
                                ptAB[:, 512:1024],
                                start=first, stop=last, tile_position=(0, 64),
                            )
                            if last:
                                nc.vector.tensor_copy(xu_ab[:], x_ab[:])
                            # --- CD side ---
                            pump(pb[kj])
                            if not last:
                                stCD = stp.tile([128, 1024], F32, tag="st",
                                                name=f"st_{quad}{qb}_{kj + 1}cd")
                                emit_qk_pair(stCD, c1, qb, kj + 1)
                            if first:
                                nc.vector.tensor_copy(psum_cd[:], ptCD[:])
                            else:
                                nc.vector.tensor_add(psum_cd[:], psum_cd[:], ptCD[:])
                            nc.tensor.matmul(
                                x_cd[0:64, :], V[:, kj, ca + 128:ca + 192],
                                ptCD[:, 0:512],
                                start=first, stop=last, tile_position=(0, 0),
                            )
                            nc.tensor.matmul(
                                x_cd[64:128, :], V[:, kj, ca + 192:ca + 256],
                                ptCD[:, 512:1024],
                                start=first, stop=last, tile_position=(0, 64),
                            )
                            if last:
                                nc.vector.tensor_copy(xu_cd[:], x_cd[:])
                        # ---- qb finale: denominators from ptsum ----
                        dd = scrp.tile([128, 512], F32, tag="ps", name=f"dd_{quad}{qb}")
                        for h, ps_sl in enumerate(
                            (psum_ab[:, 0:512], psum_ab[:, 512:1024],
                             psum_cd[:, 0:512], psum_cd[:, 512:1024])
                        ):
                            nc.tensor.matmul(
                                dd[32 * h:32 * h + 32, :], ones64[:, 0:32],
                                ps_sl, start=True, stop=True,
                                tile_position=(0, 32 * h),
                            )
                        r32 = nrm.tile([128, 512], F32, tag="r32", name=f"r_{quad}{qb}")
                        nc.vector.reciprocal_approx_fast(r32[:], dd[:])
                        # replicate each head's recip row-group 32 -> 64 parts
                        # (gpsimd-issued DMAs keep the Sync queue out of the
                        # normalization chain)
                        RAB = nrm.tile([128, 512], F32, tag="RAB", name=f"Rab_{quad}{qb}")
                        RCD = nrm.tile([128, 512], F32, tag="RCD", name=f"Rcd_{quad}{qb}")
                        for h in range(2):
                            src = r32[32 * h:32 * h + 32, :]
                            nc.gpsimd.dma_start(RAB[64 * h:64 * h + 32, :], src)
                            nc.gpsimd.dma_start(RAB[64 * h + 32:64 * h + 64, :], src)
                            src = r32[64 + 32 * h:64 + 32 * h + 32, :]
                            nc.gpsimd.dma_start(RCD[64 * h:64 * h + 32, :], src)
                            nc.gpsimd.dma_start(RCD[64 * h + 32:64 * h + 64, :], src)
                        qbsl = slice(qb * 512, (qb + 1) * 512)
                        nc.vector.tensor_mul(xT[:, c0, qbsl], xu_ab[:], RAB[:])
                        nc.vector.tensor_mul(xT[:, c1, qbsl], xu_cd[:], RCD[:])
                        # enqueue outproj for this qb once quad B's half exists
                        if quad == 1:
                            tail_tags = [(scrp, "ps"), (scrp, "ps"),
                                         (accp, "xab"), (accp, "xcd")]
                            for i, (it, oc) in enumerate(
                                (it, oc)
                                for it in range(qb * 4, qb * 4 + 4)
                                for oc in range(2)
                            ):
                                pt_sel = tail_tags[i % 4] if qb == 3 else None
                                fillers.append(
                                    lambda it=it, oc=oc, pt_sel=pt_sel:
                                        outproj_unit(it, oc, pt_sel))

            # ---- drain remaining fillers (outproj tail) ----
            with nc.named_scope("tail"):
                pump(len(fillers))

    nc.finalize()
    return nc


def _get_nc():
    if "nc" not in _CACHE:
        _CACHE["nc"] = _build()
    return _CACHE["nc"]


def _swz_x(x):
    # x [N, C] -> x^T [C, N] chunked as [4 row-chunks, 128 part, 8 ktile, 512]
    xT = np.asarray(x, np.float32).T.astype(np.float16)          # [C, N]
    return np.ascontiguousarray(
        xT.reshape(KT_TILES, 128, 4, 512).transpose(2, 1, 0, 3))


def _swz_w_qk(w):
    # w [C, HC_slice] -> [128 part, 4 qc-tile, 8 ktile, 128]
    w = np.asarray(w, np.float32).astype(np.float16)
    return np.ascontiguousarray(
        w.reshape(KT_TILES, 128, 4, 128).transpose(1, 2, 0, 3))


def _swz_w(w):
    # w [C, HC_slice] -> [128 part, 8 ktile, cols]
    w = np.asarray(w, np.float32).astype(np.float16)
    return np.ascontiguousarray(
        w.reshape(KT_TILES, 128, w.shape[1]).transpose(1, 0, 2))


def _make_in_maps(query, key, value, Wq, bq, Wk, bk, Wv, bv, Wo):
    f = np.float32
    in_maps = []
    for core in range(8):
        b, hg = divmod(core, 2)
        sl = slice(hg * HC, (hg + 1) * HC)
        in_maps.append({
            "xqT": _swz_x(query[b]),
            "xkT": _swz_x(key[b]),
            "xvT": _swz_x(value[b]),
            "wq": _swz_w_qk(np.asarray(Wq, f)[:, sl]),
            "wk": _swz_w_qk(np.asarray(Wk, f)[:, sl]),
            "wv": _swz_w(np.asarray(Wv, f)[:, sl]),
            "wo": np.ascontiguousarray(
                np.asarray(Wo, f)[sl, :].astype(np.float16)
                .reshape(4, 128, C).transpose(1, 0, 2)),
            "bq": np.ascontiguousarray(np.asarray(bq, f)[sl]),
            "bk": np.ascontiguousarray(np.asarray(bk, f)[sl]),
            "bv": np.ascontiguousarray(np.asarray(bv, f)[sl]),
        })
    return in_maps


def _run(inputs, trace=False, **kwargs):
    nc = _get_nc()
    in_maps = _make_in_maps(
        inputs["query"], inputs["key"], inputs["value"],
        inputs["Wq"], inputs["bq"], inputs["Wk"], inputs["bk"],
        inputs["Wv"], inputs["bv"], inputs["Wo"],
    )
    res = run_bass_kernel_spmd(nc, in_maps, core_ids=list(range(8)), trace=trace, **kwargs)
    bo = np.asarray(inputs["bo"], np.float32)
    out = np.empty((4, N, C), np.float32)
    for b in range(4):
        out[b] = (res.results[2 * b]["y"].astype(np.float32)
                  + res.results[2 * b + 1]["y"].astype(np.float32) + bo)
    return out, res


def kernel(**inputs) -> np.ndarray:
    out, _ = _run(inputs, trace=False)
    return out


# revision 10
# speedup vs baseline: 1.1069x; 1.0318x over previous
"""CrossAttention Trainium2 kernel (8 NeuronCores, Bass/Tile).

Problem: B=4, Nq=Nk=2048, DIM=1024, HEADS=16, HEAD_DIM=64, fp32.
  q = query @ Wq + bq ; k = key @ Wk + bk ; v = value @ Wv + bv
  attn = softmax(q k^T / 8) ; x = attn v ; out = x @ Wo + bo

Sharding: 8 cores = 4 batches x 2 head-groups (8 heads, 512 channels each).
Each core computes y_partial[b] = (attn-out restricted to its 512 channels) @ Wo_rows;
host sums the two partials per batch and adds bo.

Device design (v4.2):
  - ACT (scalar) does only the softmax exps (256 x [128,1024], ~1us each).
  - Softmax denominators: DVE accumulates ptsum += pt (fp16) per kj tile;
    one 4-head col-tiled ones-matmul per q-block turns ptsum into the
    denominators (no per-kj PE denominator matmuls).
  - Per-iter PE order is split into AB-side / CD-side halves so the PE
    never head-of-line blocks on the not-yet-finished CD exp:
      QK-AB(kj+1) | AV-ab(kj) | [pumpB] | QK-CD(kj+1) | AV-cd(kj) | [pumpC]
  - Cross-boundary pre-emission: the next q-block's first QK pairs are
    emitted inside the previous block's last iteration, so the exp stream
    never waits for the block finale.
  - Fillers (projection / outproj units) are split into ~0.5-1.3us
    sub-chunks so a pump never blocks the next QK by a full unit.
  - DMA priority: biases first, then the minimal first-exp set in k-tile
    pieces (projection matmuls start while the tail of the chunk is still
    in flight); bulk chunks stream in as fillers.
  - Long PE warmup on an uninitialized tile keeps HAM at 8/8 through the
    DMA lead-in.
  - y is written fp16 (halves output DMA); host sums partials in fp32.
"""

from collections import deque

import numpy as np

import concourse.bass as bass
import concourse.tile as tile
from concourse import bacc, mybir
from concourse.bass_utils import run_bass_kernel_spmd

F32 = mybir.dt.float32
F16 = mybir.dt.float16
EXP = mybir.ActivationFunctionType.Exp

N = 2048          # rows (Nq == Nk)
C = 1024          # model dim
HC = 512          # per-core channels (8 heads x 64)
HD = 64           # head dim
KT_TILES = C // 128   # 8 k-tiles over model dim
NJT = N // 128        # 16 kj tiles
NQB = 4               # q-blocks of 512
SCALE = 0.125         # HEAD_DIM ** -0.5
WARMUP_MMS = 48       # PE keep-warm matmuls during the DMA lead-in

_CACHE = {}

# Pump schedule, (quad, qb) -> per-kj sub-filler counts at the three pump
# points: A (iter top, delays next AB exp - use sparingly), B (between the
# AB and CD sides), C (end of iter).
_PUMP_A = {}
_PUMP_B = {
    (0, 0): 1, (0, 1): 1, (0, 2): 1, (0, 3): 1,
    (1, 0): 1, (1, 1): 1, (1, 2): 1, (1, 3): 1,
}
_PUMP_C = {(0, 0): 3, (0, 1): 1, (0, 2): 1, (0, 3): 1,
           (1, 0): 1, (1, 1): 1, (1, 2): 1, (1, 3): 1}


def _build():
    nc = bacc.Bacc("TRN2", target_bir_lowering=False, debug=False)

    xqT = nc.dram_tensor("xqT", [4, 128, KT_TILES, 512], F16, kind="ExternalInput")
    xkT = nc.dram_tensor("xkT", [4, 128, KT_TILES, 512], F16, kind="ExternalInput")
    xvT = nc.dram_tensor("xvT", [4, 128, KT_TILES, 512], F16, kind="ExternalInput")
    # wq/wk: [part, qc-tile, k-tile, 128] so a qc slice is contiguous
    wq = nc.dram_tensor("wq", [128, 4, KT_TILES, 128], F16, kind="ExternalInput")
    wk = nc.dram_tensor("wk", [128, 4, KT_TILES, 128], F16, kind="ExternalInput")
    wv = nc.dram_tensor("wv", [128, KT_TILES, HC], F16, kind="ExternalInput")
    wo = nc.dram_tensor("wo", [128, 4, C], F16, kind="ExternalInput")
    bq = nc.dram_tensor("bq", [HC], F32, kind="ExternalInput")
    bk = nc.dram_tensor("bk", [HC], F32, kind="ExternalInput")
    bv = nc.dram_tensor("bv", [HC], F32, kind="ExternalInput")
    y = nc.dram_tensor("y", [N, C], F16, kind="ExternalOutput")

    with tile.TileContext(nc) as tc:
        with (
            tc.tile_pool(name="persist", bufs=1) as pp,
            tc.tile_pool(name="chunks", bufs=2) as cp,
            tc.tile_pool(name="ptp", bufs=4) as ptp,
            tc.tile_pool(name="pts", bufs=2) as pts,
            tc.tile_pool(name="yop", bufs=2) as yop,
            tc.tile_pool(name="nrm", bufs=2) as nrm,
            tc.tile_pool(name="stp", bufs=2, space="PSUM") as stp,
            tc.tile_pool(name="accp", bufs=1, space="PSUM") as accp,
            tc.tile_pool(name="scrp", bufs=2, space="PSUM") as scrp,
        ):
            # ---- DMAs, strict priority order ----
            bq_sb = pp.tile([128, 4], F32)
            nc.sync.dma_start(bq_sb[:], bq.rearrange("(t p) -> p t", p=128))
            bk_sb = pp.tile([128, 4], F32)
            nc.sync.dma_start(bk_sb[:], bk.rearrange("(t p) -> p t", p=128))
            bv_sb = pp.tile([1, HC], F32)
            nc.sync.dma_start(bv_sb[:], bv.rearrange("(o c) -> o c", o=1))

            chunk = {}
            nload = [0]

            def load_chunk(stream, dram, sl, pieces=1):
                nload[0] += 1
                t = cp.tile([128, KT_TILES, 512], F16, tag=stream,
                            name=f"{stream}_{sl}_{nload[0]}")
                src = dram[sl]
                # pieces along the k-tile dim -> consumers of early k-tiles
                # unblock while the tail is still in flight
                kper = KT_TILES // pieces
                for s in range(pieces):
                    ksl = slice(s * kper, (s + 1) * kper)
                    nc.sync.dma_start(t[:, ksl, :], src[:, ksl, :])
                chunk[(stream, sl)] = t

            # critical set for the first exps, k-tile-pieced
            wq_sb = pp.tile([128, 4, KT_TILES, 128], F16)
            nc.sync.dma_start(wq_sb[:, 0], wq[:, 0])
            nc.sync.dma_start(wq_sb[:, 1], wq[:, 1])
            load_chunk("xq", xqT, 0, pieces=4)
            wk_sb = pp.tile([128, 4, KT_TILES, 128], F16)
            nc.sync.dma_start(wk_sb[:, 0], wk[:, 0])
            nc.sync.dma_start(wk_sb[:, 1], wk[:, 1])
            load_chunk("xk", xkT, 0, pieces=4)
            # then what AV(kj0..3) needs
            wv_sb = pp.tile([128, KT_TILES, HC], F16)
            nc.sync.dma_start(wv_sb[:, 0:4, :], wv[:, 0:4, :])
            nc.sync.dma_start(wv_sb[:, 4:8, :], wv[:, 4:8, :])
            load_chunk("xv", xvT, 0, pieces=2)
            # quad-B weight halves and wo
            nc.sync.dma_start(wq_sb[:, 2], wq[:, 2])
            nc.sync.dma_start(wk_sb[:, 2], wk[:, 2])
            nc.sync.dma_start(wq_sb[:, 3], wq[:, 3])
            nc.sync.dma_start(wk_sb[:, 3], wk[:, 3])
            wo_sb = pp.tile([128, 4, C], F16)
            nc.sync.dma_start(wo_sb[:], wo[:, :, :])

            bv_bc = pp.tile([128, HC], F32)
            nc.gpsimd.partition_broadcast(bv_bc[:], bv_sb[0:1, :])

            # ---- PE warm-up (contents unused) ----
            warm = pp.tile([128, 512], F16)
            nc.gpsimd.memset(warm[:], 0.125)
            wt = scrp.tile([128, 512], F32, tag="ps", name="warm_ps")
            for j in range(WARMUP_MMS):
                nc.tensor.matmul(wt[:], warm[:, 0:128], warm[:],
                                 start=True, stop=True)

            ones64 = pp.tile([128, 64], F16)
            nc.vector.memset(ones64[:], 1.0)

            # preload the exp ACT table so it doesn't stall attention entry
            exp_dump = pp.tile([1, 32], F32)
            nc.scalar.activation(exp_dump[:], ones64[0:1, 0:32], EXP, scale=0.0)

            QT = pp.tile([128, 4, N], F16)   # [ch-in-tile, qc-tile, q-row]
            KT = pp.tile([128, 4, N], F16)   # same layout as QT
            V = pp.tile([128, NJT, HC], F16)  # [kj-row, kj-tile, channel]
            xT = pp.tile([128, 4, N], F16)   # attention out, [ch, q] layout

            # ---- projection / outproj units, split into sub-fillers ----
            def qk_unit_parts(dstT, w_sb, b_sb, stream, qc, sl, pool_tag=None):
                cell = {}

                def part(i):
                    if i == 0:
                        pool, tag = pool_tag if pool_tag else (scrp, "ps")
                        cell["xc"] = chunk[(stream, sl)]
                        cell["ps"] = pool.tile([128, 512], F32, tag=tag,
                                               name=f"{stream}{qc}_{sl}_p")
                    ps, xc = cell["ps"], cell["xc"]
                    for k in range(4 * i, 4 * i + 4):
                        nc.tensor.matmul(
                            ps[:], w_sb[:, qc, k, :], xc[:, k, :],
                            start=(k == 0), stop=(k == KT_TILES - 1),
                        )
                    if i == 1:
                        nc.vector.tensor_scalar_add(
                            dstT[:, qc, sl * 512:(sl + 1) * 512], ps[:],
                            b_sb[:, qc:qc + 1])

                return [lambda i=i: part(i) for i in range(2)]

            def v_unit_parts(rc, rt):
                cell = {}

                def part(i):
                    kj = rc * 4 + rt
                    if i == 0:
                        cell["xc"] = chunk[("xv", rc)]
                        cell["ps"] = scrp.tile([128, 512], F32, tag="ps",
                                               name=f"v_{kj}_p")
                    ps, xc = cell["ps"], cell["xc"]
                    for k in range(4 * i, 4 * i + 4):
                        nc.tensor.matmul(
                            ps[:], xc[:, k, rt * 128:(rt + 1) * 128],
                            wv_sb[:, k, :],
                            start=(k == 0), stop=(k == KT_TILES - 1),
                        )
                    if i == 1:
                        nc.vector.tensor_add(V[:, rc * 4 + rt, :], ps[:], bv_bc[:])

                return [lambda i=i: part(i) for i in range(2)]

            ysb_cur = [None]

            def outproj_parts(it, oc, pool_tag=None):
                cell = {}

                def part(i):
                    if i == 0:
                        pool, tag = pool_tag if pool_tag else (scrp, "ps")
                        cell["ps"] = pool.tile([128, 512], F32, tag=tag,
                                               name=f"y_{it}_{oc}")
                        if oc == 0:
                            ysb_cur[0] = yop.tile([128, C], F16, tag="ysb",
                                                  name=f"ysb_{it}")
                        cell["ysb"] = ysb_cur[0]
                    yps, ysb = cell["ps"], cell["ysb"]
                    for ct in (2 * i, 2 * i + 1):
                        nc.tensor.matmul(
                            yps[:], xT[:, ct, it * 128:(it + 1) * 128],
                            wo_sb[:, ct, oc * 512:(oc + 1) * 512],
                            start=(ct == 0), stop=(ct == 3),
                        )
                    if i == 1:
                        nc.vector.tensor_copy(ysb[:, oc * 512:(oc + 1) * 512], yps[:])
                        if oc == 1:
                            nc.sync.dma_start(y[it * 128:(it + 1) * 128, :], ysb[:])

                return [lambda i=i: part(i) for i in range(2)]

            fillers = deque()

            def pump(n):
                for _ in range(n):
                    if not fillers:
                        return
                    fillers.popleft()()

            def run_unit(parts):
                for p in parts:
                    p()

            # ---- prologue: minimal proj so attention can start ----
            with nc.named_scope("prologue"):
                run_unit(qk_unit_parts(QT, wq_sb, bq_sb, "xq", 0, 0, (accp, "xab")))
                run_unit(qk_unit_parts(QT, wq_sb, bq_sb, "xq", 1, 0, (accp, "xcd")))
                run_unit(qk_unit_parts(KT, wk_sb, bk_sb, "xk", 0, 0))
                run_unit(qk_unit_parts(KT, wk_sb, bk_sb, "xk", 1, 0))

            # ---- enqueue fillers (consumed during quad-0 attn) ----
            for rt in (0, 1, 2, 3):
                fillers.extend(v_unit_parts(0, rt))
            for rc in (1, 2, 3):
                fillers.append(lambda rc=rc: load_chunk("xk", xkT, rc))
                fillers.extend(qk_unit_parts(KT, wk_sb, bk_sb, "xk", 0, rc))
                fillers.extend(qk_unit_parts(KT, wk_sb, bk_sb, "xk", 1, rc))
                fillers.append(lambda rc=rc: load_chunk("xv", xvT, rc))
                for rt in range(4):
                    fillers.extend(v_unit_parts(rc, rt))
            fillers.append(lambda: load_chunk("xq", xqT, 1))
            fillers.extend(qk_unit_parts(QT, wq_sb, bq_sb, "xq", 0, 1))
            fillers.extend(qk_unit_parts(QT, wq_sb, bq_sb, "xq", 1, 1))
            for qb, rcs in ((2, (0, 1)), (3, (2, 3))):
                fillers.append(lambda qb=qb: load_chunk("xq", xqT, qb))
                fillers.extend(qk_unit_parts(QT, wq_sb, bq_sb, "xq", 0, qb))
                fillers.extend(qk_unit_parts(QT, wq_sb, bq_sb, "xq", 1, qb))
                for rc in rcs:
                    fillers.append(lambda rc=rc: load_chunk("xk", xkT, rc))
                    fillers.extend(qk_unit_parts(KT, wk_sb, bk_sb, "xk", 2, rc))
                    fillers.extend(qk_unit_parts(KT, wk_sb, bk_sb, "xk", 3, rc))
            # quad-B Q projections, all pre-quad1 (quad1 slack is reserved
            # for the outproj stream)
            for qb in range(4):
                fillers.append(lambda qb=qb: load_chunk("xq", xqT, qb))
                fillers.extend(qk_unit_parts(QT, wq_sb, bq_sb, "xq", 2, qb))
                fillers.extend(qk_unit_parts(QT, wq_sb, bq_sb, "xq", 3, qb))

            # ---- attention ----
            def emit_qk_pair(st, c, qb, kj):
                kjsl = slice(kj * 128, (kj + 1) * 128)
                qbsl = slice(qb * 512, (qb + 1) * 512)
                nc.tensor.matmul(
                    st[:, 0:512], KT[0:64, c, kjsl], QT[0:64, c, qbsl],
                    start=True, stop=True, tile_position=(0, 0),
                )
                nc.tensor.matmul(
                    st[:, 512:1024], KT[64:128, c, kjsl], QT[64:128, c, qbsl],
                    start=True, stop=True, tile_position=(64, 0),
                )

            blocks = [(quad, qb) for quad in range(2) for qb in range(NQB)]
            nst = {}

            def new_st(quad, qb, kj, sfx):
                return stp.tile([128, 1024], F32, tag="st",
                                name=f"st_{quad}{qb}_{kj}{sfx}")

            for bi, (quad, qb) in enumerate(blocks):
                c0, c1 = 2 * quad, 2 * quad + 1
                ca = quad * 256
                nxt = blocks[bi + 1] if bi + 1 < len(blocks) else None
                with nc.named_scope(f"attn{quad}{qb}"):
                    pa = _PUMP_A.get((quad, qb), 0)
                    pb = _PUMP_B.get((quad, qb), 0)
                    pc = _PUMP_C.get((quad, qb), 0)
                    x_ab = accp.tile([128, 512], F32, tag="xab", name=f"xab_{quad}{qb}")
                    x_cd = accp.tile([128, 512], F32, tag="xcd", name=f"xcd_{quad}{qb}")
                    psum_ab = pts.tile([128, 1024], F16, tag="psab",
                                       name=f"psab_{quad}{qb}")
                    psum_cd = pts.tile([128, 1024], F16, tag="pscd",
                                       name=f"pscd_{quad}{qb}")
                    if bi == 0:
                        stAB = new_st(quad, qb, 0, "ab")
                        emit_qk_pair(stAB, c0, qb, 0)
                        stCD = new_st(quad, qb, 0, "cd")
                        emit_qk_pair(stCD, c1, qb, 0)
                    else:
                        stAB, stCD = nst.pop(bi)
                    # In block 0, AV(kj) is emitted one iteration late so the
                    # V-projection fillers never sit ahead of a QK in the PE
                    # queue while their xv DMA is still in flight.
                    lag = 1 if bi == 0 else 0
                    pt_ab, pt_cd = {}, {}
                    xu = {}

                    def av_ab(k, quad=quad, qb=qb, ca=ca, x_ab=x_ab,
                              pt_ab=pt_ab, xu=xu):
                        first, last = (k == 0), (k == NJT - 1)
                        pt = pt_ab[k]
                        nc.tensor.matmul(
                            x_ab[0:64, :], V[:, k, ca:ca + 64], pt[:, 0:512],
                            start=first, stop=last, tile_position=(0, 0),
                        )
                        nc.tensor.matmul(
                            x_ab[64:128, :], V[:, k, ca + 64:ca + 128],
                            pt[:, 512:1024],
                            start=first, stop=last, tile_position=(0, 64),
                        )
                        if last:
                            nc.vector.tensor_copy(xu["ab"][:], x_ab[:])

                    def av_cd(k, quad=quad, qb=qb, ca=ca, x_cd=x_cd,
                              pt_cd=pt_cd, xu=xu):
                        first, last = (k == 0), (k == NJT - 1)
                        pt = pt_cd[k]
                        nc.tensor.matmul(
                            x_cd[0:64, :], V[:, k, ca + 128:ca + 192],
                            pt[:, 0:512],
                            start=first, stop=last, tile_position=(0, 0),
                        )
                        nc.tensor.matmul(
                            x_cd[64:128, :], V[:, k, ca + 192:ca + 256],
                            pt[:, 512:1024],
                            start=first, stop=last, tile_position=(0, 64),
                        )
                        if last:
                            nc.vector.tensor_copy(xu["cd"][:], x_cd[:])

                    for kj in range(NJT):
                        first, last = (kj == 0), (kj == NJT - 1)
                        pump(pa)
                        ptAB = ptp.tile([128, 1024], F16, tag="pt",
                                        name=f"pt_{quad}{qb}{kj}ab")
                        nc.scalar.activation(ptAB[:], stAB[:], EXP, scale=SCALE)
                        pt_ab[kj] = ptAB
                        ptCD = ptp.tile([128, 1024], F16, tag="pt",
                                        name=f"pt_{quad}{qb}{kj}cd")
                        nc.scalar.activation(ptCD[:], stCD[:], EXP, scale=SCALE)
                        pt_cd[kj] = ptCD
                        if last:
                            xu["ab"] = nrm.tile([128, 512], F32, tag="xuab",
                                                name=f"xu_{quad}{qb}a")
                            xu["cd"] = nrm.tile([128, 512], F32, tag="xucd",
                                                name=f"xu_{quad}{qb}c")
                        # --- AB side: next QK, ptsum, AV ---
                        if not last:
                            stAB = new_st(quad, qb, kj + 1, "ab")
                            emit_qk_pair(stAB, c0, qb, kj + 1)
                        elif nxt is not None:
                            stABn = new_st(nxt[0], nxt[1], 0, "ab")
                            emit_qk_pair(stABn, 2 * nxt[0], nxt[1], 0)
                        if first:
                            nc.vector.tensor_copy(psum_ab[:], ptAB[:])
                        else:
                            nc.vector.tensor_add(psum_ab[:], psum_ab[:], ptAB[:])
                        if kj - lag >= 0:
                            av_ab(kj - lag)
                        if last and lag:
                            av_ab(NJT - 1)
                        # --- CD side ---
                        pump(pb)
                        if not last:
                            stCD = new_st(quad, qb, kj + 1, "cd")
                            emit_qk_pair(stCD, c1, qb, kj + 1)
                        elif nxt is not None:
                            stCDn = new_st(nxt[0], nxt[1], 0, "cd")
                            emit_qk_pair(stCDn, 2 * nxt[0] + 1, nxt[1], 0)
                            nst[bi + 1] = (stABn, stCDn)
                        if first:
                            nc.vector.tensor_copy(psum_cd[:], ptCD[:])
                        else:
                            nc.vector.tensor_add(psum_cd[:], psum_cd[:], ptCD[:])
                        if kj - lag >= 0:
                            av_cd(kj - lag)
                        if last and lag:
                            av_cd(NJT - 1)
                        pump(pc)
                    # ---- block finale: denominators from ptsum ----
                    dd = scrp.tile([128, 512], F32, tag="ps", name=f"dd_{quad}{qb}")
                    for h, ps_sl in enumerate(
                        (psum_ab[:, 0:512], psum_ab[:, 512:1024],
                         psum_cd[:, 0:512], psum_cd[:, 512:1024])
                    ):
                        nc.tensor.matmul(
                            dd[32 * h:32 * h + 32, :], ones64[:, 0:32],
                            ps_sl, start=True, stop=True,
                            tile_position=(0, 32 * h),
                        )
                    r32 = nrm.tile([128, 512], F32, tag="r32", name=f"r_{quad}{qb}")
                    nc.vector.reciprocal_approx_fast(r32[:], dd[:])
                    # replicate each head's recip row-group 32 -> 64 parts
                    RAB = nrm.tile([128, 512], F32, tag="RAB", name=f"Rab_{quad}{qb}")
                    RCD = nrm.tile([128, 512], F32, tag="RCD", name=f"Rcd_{quad}{qb}")
                    for h in range(2):
                        src = r32[32 * h:32 * h + 32, :]
                        nc.gpsimd.dma_start(RAB[64 * h:64 * h + 32, :], src)
                        nc.gpsimd.dma_start(RAB[64 * h + 32:64 * h + 64, :], src)
                        src = r32[64 + 32 * h:64 + 32 * h + 32, :]
                        nc.gpsimd.dma_start(RCD[64 * h:64 * h + 32, :], src)
                        nc.gpsimd.dma_start(RCD[64 * h + 32:64 * h + 64, :], src)
                    qbsl = slice(qb * 512, (qb + 1) * 512)
                    # split the normalization muls per 128-q chunk so the
                    # tail outproj can start on the first chunk immediately
                    for ich in range(4):
                        qsl = slice(qb * 512 + ich * 128, qb * 512 + ich * 128 + 128)
                        csl = slice(ich * 128, ich * 128 + 128)
                        nc.vector.tensor_mul(xT[:, c0, qsl], xu["ab"][:, csl], RAB[:, csl])
                        nc.vector.tensor_mul(xT[:, c1, qsl], xu["cd"][:, csl], RCD[:, csl])
                    if quad == 1:
                        tail_tags = [(scrp, "ps"), (scrp, "ps"),
                                     (accp, "xab"), (accp, "xcd")]
                        for i, (it, oc) in enumerate(
                            (it, oc)
                            for it in range(qb * 4, qb * 4 + 4)
                            for oc in range(2)
                        ):
                            pt_sel = tail_tags[i % 4] if qb == 3 else None
                            fillers.extend(outproj_parts(it, oc, pt_sel))

            # ---- drain remaining fillers (outproj tail) ----
            with nc.named_scope("tail"):
                pump(len(fillers))

    nc.finalize()
    return nc


def _get_nc():
    if "nc" not in _CACHE:
        _CACHE["nc"] = _build()
    return _CACHE["nc"]


def _swz_x(x):
    # x [N, C] -> x^T [C, N] chunked as [4 row-chunks, 128 part, 8 ktile, 512]
    xT = np.asarray(x, np.float32).T.astype(np.float16)          # [C, N]
    return np.ascontiguousarray(
        xT.reshape(KT_TILES, 128, 4, 512).transpose(2, 1, 0, 3))


def _swz_w_qk(w):
    # w [C, HC_slice] -> [128 part, 4 qc-tile, 8 ktile, 128]
    w = np.asarray(w, np.float32).astype(np.float16)
    return np.ascontiguousarray(
        w.reshape(KT_TILES, 128, 4, 128).transpose(1, 2, 0, 3))


def _swz_w(w):
    # w [C, HC_slice] -> [128 part, 8 ktile, cols]
    w = np.asarray(w, np.float32).astype(np.float16)
    return np.ascontiguousarray(
        w.reshape(KT_TILES, 128, w.shape[1]).transpose(1, 0, 2))


def _make_in_maps(query, key, value, Wq, bq, Wk, bk, Wv, bv, Wo):
    f = np.float32
    in_maps = []
    for core in range(8):
        b, hg = divmod(core, 2)
        sl = slice(hg * HC, (hg + 1) * HC)
        in_maps.append({
            "xqT": _swz_x(query[b]),
            "xkT": _swz_x(key[b]),
            "xvT": _swz_x(value[b]),
            "wq": _swz_w_qk(np.asarray(Wq, f)[:, sl]),
            "wk": _swz_w_qk(np.asarray(Wk, f)[:, sl]),
            "wv": _swz_w(np.asarray(Wv, f)[:, sl]),
            "wo": np.ascontiguousarray(
                np.asarray(Wo, f)[sl, :].astype(np.float16)
                .reshape(4, 128, C).transpose(1, 0, 2)),
            "bq": np.ascontiguousarray(np.asarray(bq, f)[sl]),
            "bk": np.ascontiguousarray(np.asarray(bk, f)[sl]),
            "bv": np.ascontiguousarray(np.asarray(bv, f)[sl]),
        })
    return in_maps


def _run(inputs, trace=False, **kwargs):
    nc = _get_nc()
    in_maps = _make_in_maps(
        inputs["query"], inputs["key"], inputs["value"],
        inputs["Wq"], inputs["bq"], inputs["Wk"], inputs["bk"],
        inputs["Wv"], inputs["bv"], inputs["Wo"],
    )
    res = run_bass_kernel_spmd(nc, in_maps, core_ids=list(range(8)), trace=trace, **kwargs)
    bo = np.asarray(inputs["bo"], np.float32)
    out = np.empty((4, N, C), np.float32)
    for b in range(4):
        out[b] = (res.results[2 * b]["y"].astype(np.float32)
                  + res.results[2 * b + 1]["y"].astype(np.float32) + bo)
    return out, res


def kernel(**inputs) -> np.ndarray:
    out, _ = _run(inputs, trace=False)
    return out


# revision 18
# speedup vs baseline: 1.1571x; 1.0454x over previous
"""CrossAttention Trainium2 kernel (8 NeuronCores, Bass/Tile).

Problem: B=4, Nq=Nk=2048, DIM=1024, HEADS=16, HEAD_DIM=64, fp32.
  q = query @ Wq + bq ; k = key @ Wk + bk ; v = value @ Wv + bv
  attn = softmax(q k^T / 8) ; x = attn v ; out = x @ Wo + bo

Sharding: 8 cores = 4 batches x 2 head-groups (8 heads, 512 channels each).
Each core computes y_partial[b] = (attn-out restricted to its 512 channels) @ Wo_rows;
host sums the two partials per batch and adds bo.

Device design (v4.2):
  - ACT (scalar) does only the softmax exps (256 x [128,1024], ~1us each).
  - Softmax denominators: DVE accumulates ptsum += pt (fp16) per kj tile;
    one 4-head col-tiled ones-matmul per q-block turns ptsum into the
    denominators (no per-kj PE denominator matmuls).
  - Per-iter PE order is split into AB-side / CD-side halves so the PE
    never head-of-line blocks on the not-yet-finished CD exp:
      QK-AB(kj+1) | AV-ab(kj) | [pumpB] | QK-CD(kj+1) | AV-cd(kj) | [pumpC]
  - Cross-boundary pre-emission: the next q-block's first QK pairs are
    emitted inside the previous block's last iteration, so the exp stream
    never waits for the block finale.
  - Fillers (projection / outproj units) are split into ~0.5-1.3us
    sub-chunks so a pump never blocks the next QK by a full unit.
  - DMA priority: biases first, then the minimal first-exp set in k-tile
    pieces (projection matmuls start while the tail of the chunk is still
    in flight); bulk chunks stream in as fillers.
  - Long PE warmup on an uninitialized tile keeps HAM at 8/8 through the
    DMA lead-in.
  - y is written fp16 (halves output DMA); host sums partials in fp32.
"""

from collections import deque

import numpy as np

import concourse.bass as bass
import concourse.tile as tile
from concourse import bacc, mybir
from concourse.bass_utils import run_bass_kernel_spmd

F32 = mybir.dt.float32
F16 = mybir.dt.float16
EXP = mybir.ActivationFunctionType.Exp

N = 2048          # rows (Nq == Nk)
C = 1024          # model dim
HC = 512          # per-core channels (8 heads x 64)
HD = 64           # head dim
KT_TILES = C // 128   # 8 k-tiles over model dim
NJT = N // 128        # 16 kj tiles
NQB = 4               # q-blocks of 512
SCALE = 0.125         # HEAD_DIM ** -0.5
WARMUP_MMS = 24       # PE keep-warm matmuls during the DMA lead-in

_CACHE = {}

# Pump schedule, (quad, qb) -> per-kj sub-filler counts at the three pump
# points: A (iter top, delays next AB exp - use sparingly), B (between the
# AB and CD sides), C (end of iter).
_PUMP_A = {}
_PUMP_B = {
    (0, 0): 1, (0, 1): 1, (0, 2): 1, (0, 3): 1,
    (1, 0): 1, (1, 1): 1, (1, 2): 1, (1, 3): 1,
}
_PUMP_C = {(0, 0): 3, (0, 1): 1, (0, 2): 1, (0, 3): 1}


def _build():
    nc = bacc.Bacc("TRN2", target_bir_lowering=False, debug=False)

    xqT = nc.dram_tensor("xqT", [4, 128, KT_TILES, 512], F16, kind="ExternalInput")
    xkT = nc.dram_tensor("xkT", [4, 128, KT_TILES, 512], F16, kind="ExternalInput")
    xvT = nc.dram_tensor("xvT", [4, 128, KT_TILES, 512], F16, kind="ExternalInput")
    # wq/wk: [part, qc-tile, k-tile, 128] so a qc slice is contiguous
    wq = nc.dram_tensor("wq", [128, 4, KT_TILES, 128], F16, kind="ExternalInput")
    wk = nc.dram_tensor("wk", [128, 4, KT_TILES, 128], F16, kind="ExternalInput")
    wv = nc.dram_tensor("wv", [128, KT_TILES, HC], F16, kind="ExternalInput")
    wo = nc.dram_tensor("wo", [128, 4, C], F16, kind="ExternalInput")
    bq = nc.dram_tensor("bq", [HC], F32, kind="ExternalInput")
    bk = nc.dram_tensor("bk", [HC], F32, kind="ExternalInput")
    bv = nc.dram_tensor("bv", [HC], F32, kind="ExternalInput")
    y = nc.dram_tensor("y", [N, C], F16, kind="ExternalOutput")

    with tile.TileContext(nc) as tc:
        with (
            tc.tile_pool(name="persist", bufs=1) as pp,
            tc.tile_pool(name="chunks", bufs=2) as cp,
            tc.tile_pool(name="ptp", bufs=4) as ptp,
            tc.tile_pool(name="pts", bufs=2) as pts,
            tc.tile_pool(name="yop", bufs=2) as yop,
            tc.tile_pool(name="nrm", bufs=2) as nrm,
            tc.tile_pool(name="stp", bufs=2, space="PSUM") as stp,
            tc.tile_pool(name="accp", bufs=1, space="PSUM") as accp,
            tc.tile_pool(name="scrp", bufs=2, space="PSUM") as scrp,
        ):
            # ---- DMAs, strict priority order ----
            bq_sb = pp.tile([128, 4], F32)
            nc.sync.dma_start(bq_sb[:], bq.rearrange("(t p) -> p t", p=128))
            bk_sb = pp.tile([128, 4], F32)
            nc.sync.dma_start(bk_sb[:], bk.rearrange("(t p) -> p t", p=128))
            bv_sb = pp.tile([1, HC], F32)
            nc.sync.dma_start(bv_sb[:], bv.rearrange("(o c) -> o c", o=1))

            chunk = {}
            nload = [0]

            def load_chunk(stream, dram, sl, pieces=1):
                nload[0] += 1
                t = cp.tile([128, KT_TILES, 512], F16, tag=stream,
                            name=f"{stream}_{sl}_{nload[0]}")
                src = dram[sl]
                # pieces along the k-tile dim -> consumers of early k-tiles
                # unblock while the tail is still in flight
                kper = KT_TILES // pieces
                for s in range(pieces):
                    ksl = slice(s * kper, (s + 1) * kper)
                    nc.sync.dma_start(t[:, ksl, :], src[:, ksl, :])
                chunk[(stream, sl)] = t

            # critical set for the first exps, k-tile-pieced
            wq_sb = pp.tile([128, 4, KT_TILES, 128], F16)
            nc.sync.dma_start(wq_sb[:, 0], wq[:, 0])
            nc.sync.dma_start(wq_sb[:, 1], wq[:, 1])
            load_chunk("xq", xqT, 0, pieces=4)
            wk_sb = pp.tile([128, 4, KT_TILES, 128], F16)
            nc.sync.dma_start(wk_sb[:, 0], wk[:, 0])
            nc.sync.dma_start(wk_sb[:, 1], wk[:, 1])
            load_chunk("xk", xkT, 0, pieces=4)
            # then what AV(kj0..3) needs
            wv_sb = pp.tile([128, KT_TILES, HC], F16)
            nc.sync.dma_start(wv_sb[:, 0:4, :], wv[:, 0:4, :])
            nc.sync.dma_start(wv_sb[:, 4:8, :], wv[:, 4:8, :])
            load_chunk("xv", xvT, 0, pieces=2)
            # quad-B weight halves and wo
            nc.sync.dma_start(wq_sb[:, 2], wq[:, 2])
            nc.sync.dma_start(wk_sb[:, 2], wk[:, 2])
            nc.sync.dma_start(wq_sb[:, 3], wq[:, 3])
            nc.sync.dma_start(wk_sb[:, 3], wk[:, 3])
            wo_sb = pp.tile([128, 4, C], F16)
            nc.sync.dma_start(wo_sb[:], wo[:, :, :])

            bv_bc = pp.tile([128, HC], F32)
            nc.gpsimd.partition_broadcast(bv_bc[:], bv_sb[0:1, :])

            # ---- PE warm-up (contents unused) ----
            warm = pp.tile([128, 512], F16)
            nc.gpsimd.memset(warm[:], 0.125)
            wt = scrp.tile([128, 512], F32, tag="ps", name="warm_ps")
            for j in range(WARMUP_MMS):
                nc.tensor.matmul(wt[:], warm[:, 0:128], warm[:],
                                 start=True, stop=True)

            ones64 = pp.tile([128, 64], F16)
            nc.vector.memset(ones64[:], 1.0)

            # preload the exp ACT table so it doesn't stall attention entry
            exp_dump = pp.tile([1, 32], F32)
            nc.scalar.activation(exp_dump[:], ones64[0:1, 0:32], EXP, scale=0.0)

            QT = pp.tile([128, 4, N], F16)   # [ch-in-tile, qc-tile, q-row]
            KT = pp.tile([128, 4, N], F16)   # same layout as QT
            V = pp.tile([128, NJT, HC], F16)  # [kj-row, kj-tile, channel]
            xT = pp.tile([128, 4, N], F16)   # attention out, [ch, q] layout

            # ---- projection / outproj units, split into sub-fillers ----
            def qk_unit_parts(dstT, w_sb, b_sb, stream, qc, sl, pool_tag=None):
                cell = {}

                def part(i):
                    if i == 0:
                        pool, tag = pool_tag if pool_tag else (scrp, "ps")
                        cell["xc"] = chunk[(stream, sl)]
                        cell["ps"] = pool.tile([128, 512], F32, tag=tag,
                                               name=f"{stream}{qc}_{sl}_p")
                    ps, xc = cell["ps"], cell["xc"]
                    for k in range(4 * i, 4 * i + 4):
                        nc.tensor.matmul(
                            ps[:], w_sb[:, qc, k, :], xc[:, k, :],
                            start=(k == 0), stop=(k == KT_TILES - 1),
                        )
                    if i == 1:
                        nc.vector.tensor_scalar_add(
                            dstT[:, qc, sl * 512:(sl + 1) * 512], ps[:],
                            b_sb[:, qc:qc + 1])

                return [lambda i=i: part(i) for i in range(2)]

            def v_unit_parts(rc, rt):
                cell = {}

                def part(i):
                    kj = rc * 4 + rt
                    if i == 0:
                        cell["xc"] = chunk[("xv", rc)]
                        cell["ps"] = scrp.tile([128, 512], F32, tag="ps",
                                               name=f"v_{kj}_p")
                    ps, xc = cell["ps"], cell["xc"]
                    for k in range(4 * i, 4 * i + 4):
                        nc.tensor.matmul(
                            ps[:], xc[:, k, rt * 128:(rt + 1) * 128],
                            wv_sb[:, k, :],
                            start=(k == 0), stop=(k == KT_TILES - 1),
                        )
                    if i == 1:
                        nc.vector.tensor_add(V[:, rc * 4 + rt, :], ps[:], bv_bc[:])

                return [lambda i=i: part(i) for i in range(2)]

            ysb_cur = [None]

            def outproj_parts(it, oc, pool_tag=None):
                cell = {}

                def part(i):
                    if i == 0:
                        pool, tag = pool_tag if pool_tag else (scrp, "ps")
                        cell["ps"] = pool.tile([128, 512], F32, tag=tag,
                                               name=f"y_{it}_{oc}")
                        if oc == 0:
                            ysb_cur[0] = yop.tile([128, C], F16, tag="ysb",
                                                  name=f"ysb_{it}")
                        cell["ysb"] = ysb_cur[0]
                    yps, ysb = cell["ps"], cell["ysb"]
                    for ct in (2 * i, 2 * i + 1):
                        nc.tensor.matmul(
                            yps[:], xT[:, ct, it * 128:(it + 1) * 128],
                            wo_sb[:, ct, oc * 512:(oc + 1) * 512],
                            start=(ct == 0), stop=(ct == 3),
                        )
                    if i == 1:
                        nc.vector.tensor_copy(ysb[:, oc * 512:(oc + 1) * 512], yps[:])
                        if oc == 1:
                            nc.sync.dma_start(y[it * 128:(it + 1) * 128, :], ysb[:])

                return [lambda i=i: part(i) for i in range(2)]

            fillers = deque()

            def pump(n):
                for _ in range(n):
                    if not fillers:
                        return
                    fillers.popleft()()

            def run_unit(parts):
                for p in parts:
                    p()

            def emit_qk_pair(st, c, qb, kj):
                kjsl = slice(kj * 128, (kj + 1) * 128)
                qbsl = slice(qb * 512, (qb + 1) * 512)
                nc.tensor.matmul(
                    st[:, 0:512], KT[0:64, c, kjsl], QT[0:64, c, qbsl],
                    start=True, stop=True, tile_position=(0, 0),
                )
                nc.tensor.matmul(
                    st[:, 512:1024], KT[64:128, c, kjsl], QT[64:128, c, qbsl],
                    start=True, stop=True, tile_position=(64, 0),
                )

            def new_st(quad, qb, kj, sfx):
                return stp.tile([128, 1024], F32, tag="st",
                                name=f"st_{quad}{qb}_{kj}{sfx}")

            # ---- prologue: minimal proj so attention can start ----
            st0ab_cell = [None]
            with nc.named_scope("prologue"):
                run_unit(qk_unit_parts(QT, wq_sb, bq_sb, "xq", 0, 0, (accp, "xab")))
                run_unit(qk_unit_parts(QT, wq_sb, bq_sb, "xq", 1, 0, (accp, "xcd")))
                run_unit(qk_unit_parts(KT, wk_sb, bk_sb, "xk", 0, 0))
                # first AB score pair can go as soon as K-qc0 exists; the
                # K-qc1 unit then runs behind it without delaying exp 0
                st0ab = new_st(0, 0, 0, "ab")
                emit_qk_pair(st0ab, 0, 0, 0)
                st0ab_cell[0] = st0ab
                run_unit(qk_unit_parts(KT, wk_sb, bk_sb, "xk", 1, 0))

            # ---- enqueue fillers (consumed during quad-0 attn) ----
            for rt in (0, 1, 2, 3):
                fillers.extend(v_unit_parts(0, rt))
            for rc in (1, 2, 3):
                fillers.append(lambda rc=rc: load_chunk("xk", xkT, rc))
                fillers.extend(qk_unit_parts(KT, wk_sb, bk_sb, "xk", 0, rc))
                fillers.extend(qk_unit_parts(KT, wk_sb, bk_sb, "xk", 1, rc))
                fillers.append(lambda rc=rc: load_chunk("xv", xvT, rc))
                for rt in range(4):
                    fillers.extend(v_unit_parts(rc, rt))
            fillers.append(lambda: load_chunk("xq", xqT, 1))
            fillers.extend(qk_unit_parts(QT, wq_sb, bq_sb, "xq", 0, 1))
            fillers.extend(qk_unit_parts(QT, wq_sb, bq_sb, "xq", 1, 1))
            for qb, rcs in ((2, (0, 1)), (3, (2, 3))):
                fillers.append(lambda qb=qb: load_chunk("xq", xqT, qb))
                fillers.extend(qk_unit_parts(QT, wq_sb, bq_sb, "xq", 0, qb))
                fillers.extend(qk_unit_parts(QT, wq_sb, bq_sb, "xq", 1, qb))
                for rc in rcs:
                    fillers.append(lambda rc=rc: load_chunk("xk", xkT, rc))
                    fillers.extend(qk_unit_parts(KT, wk_sb, bk_sb, "xk", 2, rc))
                    fillers.extend(qk_unit_parts(KT, wk_sb, bk_sb, "xk", 3, rc))
            # quad-B Q projections, all pre-quad1 (quad1 slack is reserved
            # for the outproj stream)
            for qb in range(4):
                fillers.append(lambda qb=qb: load_chunk("xq", xqT, qb))
                fillers.extend(qk_unit_parts(QT, wq_sb, bq_sb, "xq", 2, qb))
                fillers.extend(qk_unit_parts(QT, wq_sb, bq_sb, "xq", 3, qb))

            # ---- attention ----
            blocks = [(quad, qb) for quad in range(2) for qb in range(NQB)]
            nst = {}

            for bi, (quad, qb) in enumerate(blocks):
                c0, c1 = 2 * quad, 2 * quad + 1
                ca = quad * 256
                nxt = blocks[bi + 1] if bi + 1 < len(blocks) else None
                with nc.named_scope(f"attn{quad}{qb}"):
                    pa = _PUMP_A.get((quad, qb), 0)
                    pb = _PUMP_B.get((quad, qb), 0)
                    pc = _PUMP_C.get((quad, qb), 0)
                    x_ab = accp.tile([128, 512], F32, tag="xab", name=f"xab_{quad}{qb}")
                    x_cd = accp.tile([128, 512], F32, tag="xcd", name=f"xcd_{quad}{qb}")
                    psum_ab = pts.tile([128, 1024], F16, tag="psab",
                                       name=f"psab_{quad}{qb}")
                    psum_cd = pts.tile([128, 1024], F16, tag="pscd",
                                       name=f"pscd_{quad}{qb}")
                    if bi == 0:
                        stAB = st0ab_cell[0]
                        stCD = new_st(quad, qb, 0, "cd")
                        emit_qk_pair(stCD, c1, qb, 0)
                    else:
                        stAB, stCD = nst.pop(bi)
                    # In block 0, AV(kj) is emitted one iteration late so the
                    # V-projection fillers never sit ahead of a QK in the PE
                    # queue while their xv DMA is still in flight.
                    lag = 1 if bi == 0 else 0
                    pt_ab, pt_cd = {}, {}
                    xu = {}

                    def av_ab(k, quad=quad, qb=qb, ca=ca, x_ab=x_ab,
                              pt_ab=pt_ab, xu=xu):
                        first, last = (k == 0), (k == NJT - 1)
                        pt = pt_ab[k]
                        nc.tensor.matmul(
                            x_ab[0:64, :], V[:, k, ca:ca + 64], pt[:, 0:512],
                            start=first, stop=last, tile_position=(0, 0),
                        )
                        nc.tensor.matmul(
                            x_ab[64:128, :], V[:, k, ca + 64:ca + 128],
                            pt[:, 512:1024],
                            start=first, stop=last, tile_position=(0, 64),
                        )
                        if last:
                            nc.vector.tensor_copy(xu["ab"][:], x_ab[:])

                    def av_cd(k, quad=quad, qb=qb, ca=ca, x_cd=x_cd,
                              pt_cd=pt_cd, xu=xu):
                        first, last = (k == 0), (k == NJT - 1)
                        pt = pt_cd[k]
                        nc.tensor.matmul(
                            x_cd[0:64, :], V[:, k, ca + 128:ca + 192],
                            pt[:, 0:512],
                            start=first, stop=last, tile_position=(0, 0),
                        )
                        nc.tensor.matmul(
                            x_cd[64:128, :], V[:, k, ca + 192:ca + 256],
                            pt[:, 512:1024],
                            start=first, stop=last, tile_position=(0, 64),
                        )
                        if last:
                            nc.vector.tensor_copy(xu["cd"][:], x_cd[:])

                    for kj in range(NJT):
                        first, last = (kj == 0), (kj == NJT - 1)
                        pump(pa)
                        ptAB = ptp.tile([128, 1024], F16, tag="pt",
                                        name=f"pt_{quad}{qb}{kj}ab")
                        nc.scalar.activation(ptAB[:], stAB[:], EXP, scale=SCALE)
                        pt_ab[kj] = ptAB
                        ptCD = ptp.tile([128, 1024], F16, tag="pt",
                                        name=f"pt_{quad}{qb}{kj}cd")
                        nc.scalar.activation(ptCD[:], stCD[:], EXP, scale=SCALE)
                        pt_cd[kj] = ptCD
                        if last:
                            xu["ab"] = nrm.tile([128, 512], F32, tag="xuab",
                                                name=f"xu_{quad}{qb}a")
                            xu["cd"] = nrm.tile([128, 512], F32, tag="xucd",
                                                name=f"xu_{quad}{qb}c")
                        # --- AB side: next QK, ptsum, AV ---
                        if not last:
                            stAB = new_st(quad, qb, kj + 1, "ab")
                            emit_qk_pair(stAB, c0, qb, kj + 1)
                        elif nxt is not None:
                            stABn = new_st(nxt[0], nxt[1], 0, "ab")
                            emit_qk_pair(stABn, 2 * nxt[0], nxt[1], 0)
                        if first:
                            nc.vector.tensor_copy(psum_ab[:], ptAB[:])
                        else:
                            nc.vector.tensor_add(psum_ab[:], psum_ab[:], ptAB[:])
                        if kj - lag >= 0:
                            av_ab(kj - lag)
                        if last and lag:
                            av_ab(NJT - 1)
                        # --- CD side ---
                        pump(pb)
                        if not last:
                            stCD = new_st(quad, qb, kj + 1, "cd")
                            emit_qk_pair(stCD, c1, qb, kj + 1)
                        elif nxt is not None:
                            stCDn = new_st(nxt[0], nxt[1], 0, "cd")
                            emit_qk_pair(stCDn, 2 * nxt[0] + 1, nxt[1], 0)
                            nst[bi + 1] = (stABn, stCDn)
                        if first:
                            nc.vector.tensor_copy(psum_cd[:], ptCD[:])
                        else:
                            nc.vector.tensor_add(psum_cd[:], psum_cd[:], ptCD[:])
                        if kj - lag >= 0:
                            av_cd(kj - lag)
                        if last and lag:
                            av_cd(NJT - 1)
                        pump(pc)
                    # ---- block finale: denominators from ptsum ----
                    # M=64 ones-matmuls write each head's denominator row
                    # already replicated across its 64 channel partitions,
                    # so the reciprocal directly produces RAB/RCD.
                    dd1 = scrp.tile([128, 512], F32, tag="ps", name=f"dd1_{quad}{qb}")
                    dd2 = scrp.tile([128, 512], F32, tag="ps", name=f"dd2_{quad}{qb}")
                    for ddt, psum in ((dd1, psum_ab), (dd2, psum_cd)):
                        nc.tensor.matmul(
                            ddt[0:64, :], ones64[:, 0:64], psum[:, 0:512],
                            start=True, stop=True, tile_position=(0, 0),
                        )
                        nc.tensor.matmul(
                            ddt[64:128, :], ones64[:, 0:64], psum[:, 512:1024],
                            start=True, stop=True, tile_position=(0, 64),
                        )
                    RAB = nrm.tile([128, 512], F32, tag="RAB", name=f"Rab_{quad}{qb}")
                    RCD = nrm.tile([128, 512], F32, tag="RCD", name=f"Rcd_{quad}{qb}")
                    nc.vector.reciprocal_approx_fast(RAB[:], dd1[:])
                    nc.vector.reciprocal_approx_fast(RCD[:], dd2[:])
                    # split the normalization muls per 128-q chunk so the
                    # tail outproj can start on the first chunk immediately
                    for ich in range(4):
                        qsl = slice(qb * 512 + ich * 128, qb * 512 + ich * 128 + 128)
                        csl = slice(ich * 128, ich * 128 + 128)
                        nc.vector.tensor_mul(xT[:, c0, qsl], xu["ab"][:, csl], RAB[:, csl])
                        nc.vector.tensor_mul(xT[:, c1, qsl], xu["cd"][:, csl], RCD[:, csl])
                    if quad == 1:
                        tail_tags = [(scrp, "ps"), (scrp, "ps"),
                                     (accp, "xab"), (accp, "xcd")]
                        for i, (it, oc) in enumerate(
                            (it, oc)
                            for it in range(qb * 4, qb * 4 + 4)
                            for oc in range(2)
                        ):
                            pt_sel = tail_tags[i % 4] if qb == 3 else None
                            fillers.extend(outproj_parts(it, oc, pt_sel))

            # ---- drain remaining fillers (outproj tail) ----
            with nc.named_scope("tail"):
                pump(len(fillers))

    nc.finalize()
    return nc


def _get_nc():
    if "nc" not in _CACHE:
        _CACHE["nc"] = _build()
    return _CACHE["nc"]


def _swz_x(x):
    # x [N, C] -> x^T [C, N] chunked as [4 row-chunks, 128 part, 8 ktile, 512]
    xT = np.asarray(x, np.float32).T.astype(np.float16)          # [C, N]
    return np.ascontiguousarray(
        xT.reshape(KT_TILES, 128, 4, 512).transpose(2, 1, 0, 3))


def _swz_w_qk(w):
    # w [C, HC_slice] -> [128 part, 4 qc-tile, 8 ktile, 128]
    w = np.asarray(w, np.float32).astype(np.float16)
    return np.ascontiguousarray(
        w.reshape(KT_TILES, 128, 4, 128).transpose(1, 2, 0, 3))


def _swz_w(w):
    # w [C, HC_slice] -> [128 part, 8 ktile, cols]
    w = np.asarray(w, np.float32).astype(np.float16)
    return np.ascontiguousarray(
        w.reshape(KT_TILES, 128, w.shape[1]).transpose(1, 0, 2))


def _make_in_maps(query, key, value, Wq, bq, Wk, bk, Wv, bv, Wo):
    f = np.float32
    in_maps = []
    for core in range(8):
        b, hg = divmod(core, 2)
        sl = slice(hg * HC, (hg + 1) * HC)
        in_maps.append({
            "xqT": _swz_x(query[b]),
            "xkT": _swz_x(key[b]),
            "xvT": _swz_x(value[b]),
            "wq": _swz_w_qk(np.asarray(Wq, f)[:, sl]),
            "wk": _swz_w_qk(np.asarray(Wk, f)[:, sl]),
            "wv": _swz_w(np.asarray(Wv, f)[:, sl]),
            "wo": np.ascontiguousarray(
                np.asarray(Wo, f)[sl, :].astype(np.float16)
                .reshape(4, 128, C).transpose(1, 0, 2)),
            "bq": np.ascontiguousarray(np.asarray(bq, f)[sl]),
            "bk": np.ascontiguousarray(np.asarray(bk, f)[sl]),
            "bv": np.ascontiguousarray(np.asarray(bv, f)[sl]),
        })
    return in_maps


def _run(inputs, trace=False, **kwargs):
    nc = _get_nc()
    in_maps = _make_in_maps(
        inputs["query"], inputs["key"], inputs["value"],
        inputs["Wq"], inputs["bq"], inputs["Wk"], inputs["bk"],
        inputs["Wv"], inputs["bv"], inputs["Wo"],
    )
    res = run_bass_kernel_spmd(nc, in_maps, core_ids=list(range(8)), trace=trace, **kwargs)
    bo = np.asarray(inputs["bo"], np.float32)
    out = np.empty((4, N, C), np.float32)
    for b in range(4):
        out[b] = (res.results[2 * b]["y"].astype(np.float32)
                  + res.results[2 * b + 1]["y"].astype(np.float32) + bo)
    return out, res


def kernel(**inputs) -> np.ndarray:
    out, _ = _run(inputs, trace=False)
    return out
